# revision 6
# baseline (speedup 1.0000x reference)
"""Trainium2 Bass kernel for nn_ActorNetwork (2-layer LSTM [T=4,H=64] + 3-layer
MLP + log_softmax over a batch of 131072 13-dim states).

Strategy: pure data parallel over 8 NeuronCores (16384 samples/core).
On-chip layout is feature-major (gate-major): gates/hidden units live on SBUF
partitions, samples on the free axis. Two 512-sample subtiles ("A" at
partitions 0:64, "B" at 64:128) are pair-packed so every ACT/DVE op runs with
all 128 partitions busy. Matmuls are bf16 (fp32 PSUM accumulation); the cell
state c and the softmax tail stay fp32.
"""

import numpy as np
import ml_dtypes

import concourse.bass as bass
import concourse.mybir as mybir
from concourse.tile import TileContext
from concourse.bass_utils import run_bass_kernel_spmd
from concourse.vector_clock import ScopedClock
import concourse.tile as _tile_mod

BF16 = mybir.dt.bfloat16
F32 = mybir.dt.float32
AF = mybir.ActivationFunctionType

P = 128
FD = 512          # samples per subtile (= matmul free dim = one psum bank)
H = 64
NCORES = 8
B_TOTAL = 131072
B_CORE = B_TOTAL // NCORES          # 16384
NCHUNK = B_CORE // FD               # 32 subtiles/core
NPAIR = NCHUNK // 2                 # 16 pairs/core

# psum column region r -> PyTorch gate row range (gate order i, f, g, o)
# regions ordered [i, f, o, g]
GSLICE = [slice(0, 64), slice(64, 128), slice(192, 256), slice(128, 192)]

# ---------------------------------------------------------------------------
# walrus workaround: the TileContext tail drain may carry more sem waits than
# walrus codegen accepts for a CTRL instruction; split them across multiple
# sequencer drains (identical semantics: same engine, serial execution).
_WAIT_LIMIT = 1


def _split_excess_waits(nc, limit=_WAIT_LIMIT):
    """walrus in this env rejects instructions carrying more than one sync
    wait; hoist excess waits onto same-engine nops inserted immediately
    before the offending instruction (identical engine-stream semantics)."""
    for f in nc.m.functions:
        for bb in f.blocks:
            snapshot = list(bb.instructions)
            out = []
            changed = False
            for inst in snapshot:
                si = getattr(inst, "sync_info", None)
                waits = list(si.on_wait) if si is not None else []
                if len(waits) > limit:
                    changed = True
                    extra, keep = waits[:-limit], waits[-limit:]
                    for w in extra:
                        b = nc.engines[inst.engine].nop(
                            nofuse=True, hint="wsplit"
                        )
                        ni = b.ins
                        cb = nc.cur_bb.bb
                        cb.instructions.remove(ni)
                        ni.sync_info = mybir.SyncInfo(
                            on_wait=[w], on_update=[]
                        )
                        out.append(ni)
                    inst.sync_info = mybir.SyncInfo(
                        on_wait=keep, on_update=list(si.on_update)
                    )
                out.append(inst)
            if changed:
                bb.instructions[:] = out


def _patched_drain_and_barrier(self, tick_clock, wait_clock):
    nc = self.nc
    drain_inst = nc.sync.drain()
    wait_clock.add_sem_waits(
        drain_inst.ins, ScopedClock({None: tick_clock.global_clock})
    )
    si = drain_inst.ins.sync_info
    waits = list(si.on_wait) if si is not None else []
    if len(waits) > _WAIT_LIMIT:
        drain_inst.ins.sync_info = mybir.SyncInfo(
            on_wait=waits[:_WAIT_LIMIT], on_update=list(si.on_update)
        )
        for k in range(_WAIT_LIMIT, len(waits), _WAIT_LIMIT):
            d2 = nc.sync.drain()
            d2.ins.sync_info = mybir.SyncInfo(
                on_wait=waits[k : k + _WAIT_LIMIT], on_update=[]
            )
    nc.all_engine_barrier()
    popped = nc._tile_sem_poison_stack.pop()
    assert popped is self._sem_poison
    nc.clear_and_free_semaphores(list(self.sems.allocated().values()))
    nc.all_engine_barrier()
    _split_excess_waits(nc)


_tile_mod.TileContext._drain_and_barrier = _patched_drain_and_barrier
# ---------------------------------------------------------------------------


def build_program(nchunk=NCHUNK):
    """Build the SPMD Bass program for one core processing nchunk*FD samples."""
    assert nchunk % 2 == 0
    npair = nchunk // 2
    ncols = nchunk * FD

    nc = bass.Bass("TRN2", num_devices=NCORES)

    xp = nc.declare_dram_parameter("xp", [P, ncols], BF16, isOutput=False)
    wh0_d = nc.declare_dram_parameter("wh0", [128, 256], BF16, isOutput=False)
    wx0_d = nc.declare_dram_parameter("wx0", [128, 256], BF16, isOutput=False)
    w1a_d = nc.declare_dram_parameter("w1a", [128, 256], BF16, isOutput=False)
    w1b_d = nc.declare_dram_parameter("w1b", [128, 256], BF16, isOutput=False)
    w1h_d = nc.declare_dram_parameter("w1h", [128, 30], BF16, isOutput=False)
    w1f_d = nc.declare_dram_parameter("w1f", [37, 30], BF16, isOutput=False)
    wm2_d = nc.declare_dram_parameter("wm2", [30, 10], BF16, isOutput=False)
    wm3_d = nc.declare_dram_parameter("wm3", [11, 4], BF16, isOutput=False)
    bl0_d = nc.declare_dram_parameter("bl0", [128, 4], F32, isOutput=False)
    bl1_d = nc.declare_dram_parameter("bl1", [128, 4], F32, isOutput=False)
    bm1_d = nc.declare_dram_parameter("bm1", [30, 1], F32, isOutput=False)
    bm2_d = nc.declare_dram_parameter("bm2", [10, 1], F32, isOutput=False)
    no_d = nc.declare_dram_parameter("negones", [33, 4], F32, isOutput=False)
    o4_d = nc.declare_dram_parameter("ones4", [4, 1], F32, isOutput=False)
    out_d = nc.declare_dram_parameter("out", [4, ncols], F32, isOutput=True)

    with TileContext(nc) as tc:
        with (
            tc.tile_pool(name="const", bufs=1) as const,
            tc.tile_pool(name="xpool", bufs=3) as xpool,
            tc.tile_pool(name="gp", bufs=3) as gp,
            tc.tile_pool(name="lp", bufs=2) as lp,
            tc.tile_pool(name="st", bufs=2) as st,
            tc.tile_pool(name="pers", bufs=npair) as pers,
            tc.tile_pool(name="pp", bufs=2, space="PSUM") as pp,
            tc.tile_pool(name="p2", bufs=2) as p2,
        ):
            # ---- constants -------------------------------------------------
            wh0 = const.tile([128, 256], BF16, name="wh0")
            nc.sync.dma_start(wh0[:], wh0_d[:, :])
            wx0 = const.tile([128, 256], BF16, name="wx0")
            nc.sync.dma_start(wx0[:], wx0_d[:, :])
            w1a = const.tile([128, 256], BF16, name="w1a")
            nc.sync.dma_start(w1a[:], w1a_d[:, :])
            w1b = const.tile([128, 256], BF16, name="w1b")
            nc.sync.dma_start(w1b[:], w1b_d[:, :])
            w1h = const.tile([128, 30], BF16, name="w1h")
            nc.sync.dma_start(w1h[:], w1h_d[:, :])
            w1f = const.tile([37, 30], BF16, name="w1f")
            nc.sync.dma_start(w1f[:], w1f_d[:, :])
            wm2 = const.tile([30, 10], BF16, name="wm2")
            nc.sync.dma_start(wm2[:], wm2_d[:, :])
            wm3 = const.tile([11, 4], BF16, name="wm3")
            nc.sync.dma_start(wm3[:], wm3_d[:, :])
            bl0 = const.tile([128, 4], F32, name="bl0")
            nc.sync.dma_start(bl0[:], bl0_d[:, :])
            bl1 = const.tile([128, 4], F32, name="bl1")
            nc.sync.dma_start(bl1[:], bl1_d[:, :])
            bm1 = const.tile([30, 1], F32, name="bm1")
            nc.sync.dma_start(bm1[:], bm1_d[:, :])
            bm2 = const.tile([10, 1], F32, name="bm2")
            nc.sync.dma_start(bm2[:], bm2_d[:, :])
            nones = const.tile([33, 4], F32, name="nones")
            nc.sync.dma_start(nones[:], no_d[:, :])
            ones4 = const.tile([4, 1], F32, name="ones4")
            nc.sync.dma_start(ones4[:], o4_d[:, :])
            onesbf = const.tile([1, 1024], BF16, name="onesbf")
            nc.vector.memset(onesbf[:], 1.0)

            persist = []

            # ================= phase 1: LSTM stack =========================
            for p in range(npair):
                ca = slice(2 * p * FD, (2 * p + 1) * FD)
                cb = slice((2 * p + 1) * FD, (2 * p + 2) * FD)

                xa = xpool.tile([P, FD], BF16, name="xa")
                nc.sync.dma_start(xa[:], xp[:, ca])
                xb = xpool.tile([P, FD], BF16, name="xb")
                nc.sync.dma_start(xb[:], xp[:, cb])
                ft = pers.tile([37, FD], BF16, name="ft")
                nc.sync.dma_start(ft[32:37, :], xp[8:13, ca])  # A feats
                nc.sync.dma_start(ft[0:5, :], xp[8:13, cb])    # B feats

                mlph = pers.tile([P, FD], BF16, name="mlph")
                l1a = [lp.tile([P, FD], BF16, name=f"l1a{t}") for t in range(4)]
                l1b = [lp.tile([P, FD], BF16, name=f"l1b{t}") for t in range(4)]
                c0 = st.tile([P, FD], F32, name="c0")
                c1 = st.tile([P, FD], F32, name="c1")

                for layer in (0, 1):
                    bias = bl0 if layer == 0 else bl1
                    cstate = c0 if layer == 0 else c1
                    for t in range(4):
                        ps = pp.tile([128, 2048], F32, name="ps")
                        for r in range(4):
                            gc = slice(r * 64, (r + 1) * 64)
                            regc = slice(r * FD, (r + 1) * FD)
                            if layer == 0:
                                # pack [A(0:64); B(64:128)]
                                for xtile, l1t, lo in (
                                    (xa, l1a, True),
                                    (xb, l1b, False),
                                ):
                                    orows = slice(0, 64) if lo else slice(64, 128)
                                    oap = ps[orows, regc]
                                    cpos = 0 if lo else 64
                                    if t >= 1:
                                        hr = slice(0, 64) if lo else slice(64, 128)
                                        nc.tensor.matmul(
                                            oap,
                                            lhsT=wh0[hr, gc],
                                            rhs=l1t[t - 1][hr, :],
                                            start=True,
                                            stop=False,
                                            tile_position=(0 if lo else 64, cpos),
                                        )
                                    nc.tensor.matmul(
                                        oap,
                                        lhsT=wx0[32 * t : 32 * t + 2, gc],
                                        rhs=xtile[32 * t : 32 * t + 2, :],
                                        start=(t == 0),
                                        stop=True,
                                        tile_position=(32 * t, cpos),
                                    )
                            else:
                                # pack [B(0:64); A(64:128)]
                                if t == 0:
                                    nc.tensor.matmul(
                                        ps[64:128, regc],
                                        lhsT=w1a[0:64, gc],
                                        rhs=l1a[0][0:64, :],
                                        start=True,
                                        stop=True,
                                        tile_position=(0, 64),
                                    )
                                    nc.tensor.matmul(
                                        ps[0:64, regc],
                                        lhsT=w1b[64:128, gc],
                                        rhs=l1b[0][64:128, :],
                                        start=True,
                                        stop=True,
                                        tile_position=(64, 0),
                                    )
                                else:
                                    nc.tensor.matmul(
                                        ps[64:128, regc],
                                        lhsT=w1a[:, gc],
                                        rhs=l1a[t][:, :],
                                        start=True,
                                        stop=True,
                                        tile_position=(0, 64),
                                    )
                                    nc.tensor.matmul(
                                        ps[0:64, regc],
                                        lhsT=w1b[:, gc],
                                        rhs=l1b[t][:, :],
                                        start=True,
                                        stop=True,
                                        tile_position=(0, 0),
                                    )

                        # activations (regions: 0=i, 1=f, 2=o, 3=g)
                        si = gp.tile([P, FD], F32, name="si")
                        nc.scalar.activation(
                            si[:], ps[:, 0:FD], AF.Sigmoid, bias=bias[:, 0:1]
                        )
                        if t > 0:
                            sf = gp.tile([P, FD], F32, name="sf")
                            nc.scalar.activation(
                                sf[:], ps[:, FD : 2 * FD], AF.Sigmoid,
                                bias=bias[:, 1:2],
                            )
                        so_ = gp.tile([P, FD], BF16, name="so_")
                        nc.scalar.activation(
                            so_[:], ps[:, 2 * FD : 3 * FD], AF.Sigmoid,
                            bias=bias[:, 2:3],
                        )
                        tg = gp.tile([P, FD], F32, name="tg")
                        nc.scalar.activation(
                            tg[:], ps[:, 3 * FD : 4 * FD], AF.Tanh,
                            bias=bias[:, 3:4],
                        )

                        if t == 0:
                            nc.vector.tensor_mul(cstate[:], si[:], tg[:])
                        else:
                            t1 = gp.tile([P, FD], F32, name="t1")
                            nc.vector.tensor_mul(t1[:], si[:], tg[:])
                            t2 = gp.tile([P, FD], F32, name="t2")
                            nc.vector.tensor_mul(t2[:], sf[:], cstate[:])
                            nc.vector.tensor_add(cstate[:], t1[:], t2[:])

                        tcx = gp.tile([P, FD], BF16, name="tcx")
                        nc.scalar.activation(tcx[:], cstate[:], AF.Tanh)

                        if layer == 0:
                            dlo = l1a[t][0:64, :]
                            dhi = l1b[t][64:128, :]
                        elif t < 3:
                            dlo = l1b[t + 1][0:64, :]
                            dhi = l1a[t + 1][64:128, :]
                        else:
                            dlo = mlph[0:64, :]
                            dhi = mlph[64:128, :]
                        nc.vector.tensor_mul(dlo, so_[0:64, :], tcx[0:64, :])
                        nc.vector.tensor_mul(dhi, so_[64:128, :], tcx[64:128, :])

                persist.append((mlph, ft))

            # ================= phase 2: MLP + log_softmax ===================
            for p, (mlph, ft) in enumerate(persist):
                ps = pp.tile([128, 2048], F32, name="ps")
                # layer 1 of MLP: A -> cols 0:512, B -> cols 512:1024
                nc.tensor.matmul(
                    ps[0:30, 0:512], lhsT=w1h[64:128, :], rhs=mlph[64:128, :],
                    start=True, stop=False, tile_position=(64, 0),
                )
                nc.tensor.matmul(
                    ps[0:30, 0:512], lhsT=w1f[32:37, :], rhs=ft[32:37, :],
                    start=False, stop=True, tile_position=(32, 0),
                )
                nc.tensor.matmul(
                    ps[0:30, 512:1024], lhsT=w1h[0:64, :], rhs=mlph[0:64, :],
                    start=True, stop=False, tile_position=(0, 0),
                )
                nc.tensor.matmul(
                    ps[0:30, 512:1024], lhsT=w1f[0:5, :], rhs=ft[0:5, :],
                    start=False, stop=True, tile_position=(0, 0),
                )
                m1s = p2.tile([30, 1024], BF16, name="m1s")
                nc.scalar.activation(
                    m1s[:], ps[0:30, 0:1024], AF.Relu, bias=bm1[:]
                )
                nc.tensor.matmul(
                    ps[0:10, 1024:1536], lhsT=wm2[:], rhs=m1s[:, 0:512],
                    start=True, stop=True, tile_position=(0, 0),
                )
                nc.tensor.matmul(
                    ps[0:10, 1536:2048], lhsT=wm2[:], rhs=m1s[:, 512:1024],
                    start=True, stop=True, tile_position=(0, 0),
                )
                m2s = p2.tile([11, 1024], BF16, name="m2s")
                nc.scalar.activation(
                    m2s[0:10, :], ps[0:10, 1024:2048], AF.Relu, bias=bm2[:]
                )
                nc.sync.dma_start(m2s[10:11, :], onesbf[:])
                # layer 3 (W3 + b3 via ones row), keep group open for -log(s)
                nc.tensor.matmul(
                    ps[0:4, 0:512], lhsT=wm3[:], rhs=m2s[:, 0:512],
                    start=True, stop=True, tile_position=(0, 0),
                )
                nc.tensor.matmul(
                    ps[0:4, 512:1024], lhsT=wm3[:], rhs=m2s[:, 512:1024],
                    start=True, stop=True, tile_position=(0, 0),
                )
                es = p2.tile([4, 1024], F32, name="es")
                nc.scalar.activation(es[:], ps[0:4, 0:1024], AF.Exp)
                nc.tensor.matmul(
                    ps[32:33, 1024:1536], lhsT=ones4[:], rhs=es[:, 0:512],
                    start=True, stop=True, tile_position=(0, 32),
                )
                nc.tensor.matmul(
                    ps[32:33, 1536:2048], lhsT=ones4[:], rhs=es[:, 512:1024],
                    start=True, stop=True, tile_position=(0, 32),
                )
                ls = p2.tile([33, 1024], F32, name="ls")
                nc.scalar.activation(ls[32:33, :], ps[32:33, 1024:2048], AF.Ln)
                nc.tensor.matmul(
                    ps[0:4, 0:512], lhsT=nones[32:33, :], rhs=ls[32:33, 0:512],
                    start=False, stop=True, tile_position=(32, 0),
                    skip_group_check=True,
                )
                nc.tensor.matmul(
                    ps[0:4, 512:1024], lhsT=nones[32:33, :],
                    rhs=ls[32:33, 512:1024],
                    start=False, stop=True, tile_position=(32, 0),
                    skip_group_check=True,
                )
                fo = p2.tile([4, 1024], F32, name="fo")
                nc.vector.tensor_copy(fo[:], ps[0:4, 0:1024])
                nc.sync.dma_start(out_d[:, 1024 * p : 1024 * (p + 1)], fo[:])

    return nc


def pack_weights(Wih0, Whh0, bih0, bhh0, Wih1, Whh1, bih1, bhh1,
                 W1, b1, W2, b2, W3, b3):
    bf = ml_dtypes.bfloat16
    wh0 = np.zeros((128, 256), np.float32)
    wx0 = np.zeros((128, 256), np.float32)
    w1a = np.zeros((128, 256), np.float32)
    w1b = np.zeros((128, 256), np.float32)
    for r, sl in enumerate(GSLICE):
        gc = slice(r * 64, (r + 1) * 64)
        wh0[0:64, gc] = Whh0[sl].T
        wh0[64:128, gc] = Whh0[sl].T
        for t in range(4):
            wx0[32 * t : 32 * t + 2, gc] = Wih0[sl].T
        w1a[0:64, gc] = Wih1[sl].T
        w1a[64:128, gc] = Whh1[sl].T
        w1b[0:64, gc] = Whh1[sl].T
        w1b[64:128, gc] = Wih1[sl].T
    w1h = np.zeros((128, 30), np.float32)
    w1h[0:64] = W1[:, 0:64].T
    w1h[64:128] = W1[:, 0:64].T
    w1f = np.zeros((37, 30), np.float32)
    w1f[0:5] = W1[:, 64:69].T
    w1f[32:37] = W1[:, 64:69].T
    wm2 = np.ascontiguousarray(W2.T)
    wm3 = np.zeros((11, 4), np.float32)
    wm3[0:10] = W3.T
    wm3[10] = b3
    b0 = bih0 + bhh0
    b1l = bih1 + bhh1
    bl0 = np.zeros((128, 4), np.float32)
    bl1 = np.zeros((128, 4), np.float32)
    for r, sl in enumerate(GSLICE):
        bl0[0:64, r] = b0[sl]
        bl0[64:128, r] = b0[sl]
        bl1[0:64, r] = b1l[sl]
        bl1[64:128, r] = b1l[sl]
    return {
        "wh0": wh0.astype(bf),
        "wx0": wx0.astype(bf),
        "w1a": w1a.astype(bf),
        "w1b": w1b.astype(bf),
        "w1h": w1h.astype(bf),
        "w1f": w1f.astype(bf),
        "wm2": wm2.astype(bf),
        "wm3": wm3.astype(bf),
        "bl0": bl0,
        "bl1": bl1,
        "bm1": np.ascontiguousarray(b1.reshape(30, 1), dtype=np.float32),
        "bm2": np.ascontiguousarray(b2.reshape(10, 1), dtype=np.float32),
        "negones": np.full((33, 4), -1.0, np.float32),
        "ones4": np.ones((4, 1), np.float32),
    }


def pack_x(xs):
    """xs: [n, 13] f32 -> xp [128, n] bf16 (n multiple of FD)."""
    n = xs.shape[0]
    xpv = np.zeros((128, n), np.float32)
    for t in range(4):
        xpv[32 * t] = xs[:, 2 * t]
        xpv[32 * t + 1] = xs[:, 2 * t + 1]
    xpv[8:13] = xs[:, 8:13].T
    return xpv.astype(ml_dtypes.bfloat16)


_cached = {}


def run_cores(x, weights, trace=False):
    """x: [B_TOTAL, 13] f32. Returns (out [B_TOTAL, 4] f32, BassKernelResults)."""
    key = "prog"
    if key not in _cached:
        _cached[key] = build_program(NCHUNK)
    nc = _cached[key]
    in_maps = []
    for c in range(NCORES):
        xs = x[c * B_CORE : (c + 1) * B_CORE]
        m = dict(weights)
        m["xp"] = pack_x(xs)
        in_maps.append(m)
    res = run_bass_kernel_spmd(
        nc, in_maps, core_ids=list(range(NCORES)), trace=trace
    )
    outs = [res.results[c]["out"] for c in range(NCORES)]  # [4, 16384] each
    full = np.concatenate([o.T for o in outs], axis=0)     # [B_TOTAL, 4]
    return np.ascontiguousarray(full, dtype=np.float32), res


def kernel(x, Wih0, Whh0, bih0, bhh0, Wih1, Whh1, bih1, bhh1,
           W1, b1, W2, b2, W3, b3):
    args = [np.asarray(a, dtype=np.float32) for a in (
        Wih0, Whh0, bih0, bhh0, Wih1, Whh1, bih1, bhh1, W1, b1, W2, b2, W3, b3
    )]
    weights = pack_weights(*args)
    out, _ = run_cores(np.asarray(x, dtype=np.float32), weights)
    return out


# revision 8
# speedup vs baseline: 1.2897x; 1.2897x over previous
"""Trainium2 Bass kernel for nn_ActorNetwork (2-layer LSTM [T=4,H=64] + 3-layer
MLP + log_softmax over a batch of 131072 13-dim states).

Strategy: pure data parallel over 8 NeuronCores (16384 samples/core).
On-chip layout is feature-major (gate-major): gates/hidden units live on SBUF
partitions, samples on the free axis. Two 512-sample subtiles ("A", "B") are
pair-packed on the partition axis so ACT/DVE ops run with all 128 partitions
busy, and two such pairs are emitted in lockstep so the engines pipeline
across them. Matmuls are bf16 (fp32 PSUM accumulation); the cell state c and
the softmax tail stay fp32.
"""

import numpy as np
import ml_dtypes

import concourse.bass as bass
import concourse.mybir as mybir
from concourse.tile import TileContext
from concourse.bass_utils import run_bass_kernel_spmd
from concourse.vector_clock import ScopedClock
import concourse.tile as _tile_mod

BF16 = mybir.dt.bfloat16
F32 = mybir.dt.float32
AF = mybir.ActivationFunctionType

P = 128
FD = 512          # samples per subtile (= matmul free dim = one psum bank)
H = 64
NCORES = 8
B_TOTAL = 131072
B_CORE = B_TOTAL // NCORES          # 16384
NCHUNK = B_CORE // FD               # 32 subtiles/core
NPAIR = NCHUNK // 2                 # 16 pairs/core

# psum column region r -> PyTorch gate row range (PyTorch gate order i,f,g,o)
# regions ordered [i, f, o, g] so sigmoid covers a contiguous [I|F|O] block
GSLICE = [slice(0, 64), slice(64, 128), slice(192, 256), slice(128, 192)]

# ---------------------------------------------------------------------------
# walrus workaround: this toolchain rejects instructions carrying more than
# one sync wait; split excess waits onto same-engine nops inserted right
# before the offending instruction (identical engine-stream semantics).
_WAIT_LIMIT = 1


def _split_excess_waits(nc, limit=_WAIT_LIMIT):
    for f in nc.m.functions:
        for bb in f.blocks:
            snapshot = list(bb.instructions)
            out = []
            changed = False
            for inst in snapshot:
                si = getattr(inst, "sync_info", None)
                waits = list(si.on_wait) if si is not None else []
                if len(waits) > limit:
                    changed = True
                    extra, keep = waits[:-limit], waits[-limit:]
                    for w in extra:
                        b = nc.engines[inst.engine].nop(
                            nofuse=True, hint="wsplit"
                        )
                        ni = b.ins
                        cb = nc.cur_bb.bb
                        cb.instructions.remove(ni)
                        ni.sync_info = mybir.SyncInfo(
                            on_wait=[w], on_update=[]
                        )
                        out.append(ni)
                    inst.sync_info = mybir.SyncInfo(
                        on_wait=keep, on_update=list(si.on_update)
                    )
                out.append(inst)
            if changed:
                bb.instructions[:] = out


def _patched_drain_and_barrier(self, tick_clock, wait_clock):
    nc = self.nc
    drain_inst = nc.sync.drain()
    wait_clock.add_sem_waits(
        drain_inst.ins, ScopedClock({None: tick_clock.global_clock})
    )
    si = drain_inst.ins.sync_info
    waits = list(si.on_wait) if si is not None else []
    if len(waits) > _WAIT_LIMIT:
        drain_inst.ins.sync_info = mybir.SyncInfo(
            on_wait=waits[:_WAIT_LIMIT], on_update=list(si.on_update)
        )
        for k in range(_WAIT_LIMIT, len(waits), _WAIT_LIMIT):
            d2 = nc.sync.drain()
            d2.ins.sync_info = mybir.SyncInfo(
                on_wait=waits[k : k + _WAIT_LIMIT], on_update=[]
            )
    nc.all_engine_barrier()
    popped = nc._tile_sem_poison_stack.pop()
    assert popped is self._sem_poison
    nc.clear_and_free_semaphores(list(self.sems.allocated().values()))
    nc.all_engine_barrier()
    _split_excess_waits(nc)


_tile_mod.TileContext._drain_and_barrier = _patched_drain_and_barrier
# ---------------------------------------------------------------------------


class _PairCtx:
    """SBUF tiles and per-pair state for one in-flight pair."""

    __slots__ = ("idx", "xat", "xbt", "ft", "mlph", "l1a", "l1b", "c0", "c1")


def build_program(nchunk=NCHUNK):
    """Build the SPMD Bass program for one core processing nchunk*FD samples."""
    assert nchunk % 2 == 0
    npair = nchunk // 2
    ncols = nchunk * FD

    nc = bass.Bass("TRN2", num_devices=NCORES)

    xq = nc.declare_dram_parameter("xq", [4, P, ncols], BF16, isOutput=False)
    wh0_d = nc.declare_dram_parameter("wh0", [128, 256], BF16, isOutput=False)
    wx0_d = nc.declare_dram_parameter("wx0", [128, 256], BF16, isOutput=False)
    w1a_d = nc.declare_dram_parameter("w1a", [128, 256], BF16, isOutput=False)
    w1b_d = nc.declare_dram_parameter("w1b", [128, 256], BF16, isOutput=False)
    w1h_d = nc.declare_dram_parameter("w1h", [128, 30], BF16, isOutput=False)
    w1f_d = nc.declare_dram_parameter("w1f", [37, 30], BF16, isOutput=False)
    wm2_d = nc.declare_dram_parameter("wm2", [30, 10], BF16, isOutput=False)
    wm3_d = nc.declare_dram_parameter("wm3", [11, 4], BF16, isOutput=False)
    bl1_d = nc.declare_dram_parameter("bl1", [128, 4], F32, isOutput=False)
    bm1_d = nc.declare_dram_parameter("bm1", [30, 1], F32, isOutput=False)
    bm2_d = nc.declare_dram_parameter("bm2", [10, 1], F32, isOutput=False)
    no_d = nc.declare_dram_parameter("negones", [33, 4], F32, isOutput=False)
    o4_d = nc.declare_dram_parameter("ones4", [4, 1], F32, isOutput=False)
    out_d = nc.declare_dram_parameter("out", [4, ncols], F32, isOutput=True)

    with TileContext(nc) as tc:
        with (
            tc.tile_pool(name="const", bufs=1) as const,
            tc.tile_pool(name="xpool", bufs=3) as xpool,
            tc.tile_pool(name="gp", bufs=3) as gp,
            tc.tile_pool(name="lp", bufs=3) as lp,
            tc.tile_pool(name="st", bufs=3) as st,
            tc.tile_pool(name="pers", bufs=npair) as pers,
            tc.tile_pool(name="pp", bufs=2, space="PSUM") as pp,
            tc.tile_pool(name="p2", bufs=2) as p2,
        ):
            # ---- constants -------------------------------------------------
            wh0 = const.tile([128, 256], BF16, name="wh0")
            nc.sync.dma_start(wh0[:], wh0_d[:, :])
            wx0 = const.tile([128, 256], BF16, name="wx0")
            nc.sync.dma_start(wx0[:], wx0_d[:, :])
            w1a = const.tile([128, 256], BF16, name="w1a")
            nc.sync.dma_start(w1a[:], w1a_d[:, :])
            w1b = const.tile([128, 256], BF16, name="w1b")
            nc.sync.dma_start(w1b[:], w1b_d[:, :])
            w1h = const.tile([128, 30], BF16, name="w1h")
            nc.sync.dma_start(w1h[:], w1h_d[:, :])
            w1f = const.tile([37, 30], BF16, name="w1f")
            nc.sync.dma_start(w1f[:], w1f_d[:, :])
            wm2 = const.tile([30, 10], BF16, name="wm2")
            nc.sync.dma_start(wm2[:], wm2_d[:, :])
            wm3 = const.tile([11, 4], BF16, name="wm3")
            nc.sync.dma_start(wm3[:], wm3_d[:, :])
            bl1 = const.tile([128, 4], F32, name="bl1")
            nc.sync.dma_start(bl1[:], bl1_d[:, :])
            bm1 = const.tile([30, 1], F32, name="bm1")
            nc.sync.dma_start(bm1[:], bm1_d[:, :])
            bm2 = const.tile([10, 1], F32, name="bm2")
            nc.sync.dma_start(bm2[:], bm2_d[:, :])
            nones = const.tile([33, 4], F32, name="nones")
            nc.sync.dma_start(nones[:], no_d[:, :])
            ones4 = const.tile([4, 1], F32, name="ones4")
            nc.sync.dma_start(ones4[:], o4_d[:, :])
            onesbf = const.tile([1, 1024], BF16, name="onesbf")
            nc.vector.memset(onesbf[:], 1.0)

            persist = []

            def open_pair(p):
                px = _PairCtx()
                px.idx = p
                ca = slice(2 * p * FD, (2 * p + 1) * FD)
                cb = slice((2 * p + 1) * FD, (2 * p + 2) * FD)
                px.xat = []
                px.xbt = []
                for t in range(4):
                    xa = xpool.tile([P, FD], BF16, name=f"xa{t}")
                    nc.sync.dma_start(xa[:], xq[t, :, ca])
                    px.xat.append(xa)
                    xb = xpool.tile([P, FD], BF16, name=f"xb{t}")
                    nc.sync.dma_start(xb[:], xq[t, :, cb])
                    px.xbt.append(xb)
                ft = pers.tile([37, FD], BF16, name="ft")
                nc.sync.dma_start(ft[32:37, :], xq[0, 8:13, ca])  # A feats
                nc.sync.dma_start(ft[0:5, :], xq[0, 8:13, cb])    # B feats
                px.ft = ft
                px.mlph = pers.tile([P, FD], BF16, name="mlph")
                px.l1a = [lp.tile([P, FD], BF16, name=f"l1a{t}") for t in range(4)]
                px.l1b = [lp.tile([P, FD], BF16, name=f"l1b{t}") for t in range(4)]
                px.c0 = st.tile([P, FD], F32, name="c0")
                px.c1 = st.tile([P, FD], F32, name="c1")
                return px

            def emit_step(px, layer, t):
                ps = pp.tile([128, 2048], F32, name="ps")
                for r in range(4):
                    gc = slice(r * 64, (r + 1) * 64)
                    regc = slice(r * FD, (r + 1) * FD)
                    if layer == 0:
                        # pack [A(0:64); B(64:128)]
                        for xtiles, l1t, lo in (
                            (px.xat, px.l1a, True),
                            (px.xbt, px.l1b, False),
                        ):
                            orows = slice(0, 64) if lo else slice(64, 128)
                            oap = ps[orows, regc]
                            cpos = 0 if lo else 64
                            if t >= 1:
                                hr = slice(0, 64) if lo else slice(64, 128)
                                nc.tensor.matmul(
                                    oap,
                                    lhsT=wh0[hr, gc],
                                    rhs=l1t[t - 1][hr, :],
                                    start=True,
                                    stop=False,
                                    tile_position=(0 if lo else 64, cpos),
                                )
                            # x-projection + bias ones-row (K=3), row-tiled
                            # per gate (rows 32r) so the 4 gates overlap in
                            # the PE array.
                            nc.tensor.matmul(
                                oap,
                                lhsT=wx0[32 * r : 32 * r + 3, gc],
                                rhs=xtiles[t][32 * r : 32 * r + 3, :],
                                start=(t == 0),
                                stop=True,
                                tile_position=(32 * r, cpos),
                            )
                    else:
                        # pack [B(0:64); A(64:128)]
                        if t == 0:
                            nc.tensor.matmul(
                                ps[64:128, regc],
                                lhsT=w1a[0:64, gc],
                                rhs=px.l1a[0][0:64, :],
                                start=True,
                                stop=True,
                                tile_position=(0, 64),
                            )
                            nc.tensor.matmul(
                                ps[0:64, regc],
                                lhsT=w1b[64:128, gc],
                                rhs=px.l1b[0][64:128, :],
                                start=True,
                                stop=True,
                                tile_position=(64, 0),
                            )
                        else:
                            nc.tensor.matmul(
                                ps[64:128, regc],
                                lhsT=w1a[:, gc],
                                rhs=px.l1a[t][:, :],
                                start=True,
                                stop=True,
                                tile_position=(0, 64),
                            )
                            nc.tensor.matmul(
                                ps[0:64, regc],
                                lhsT=w1b[:, gc],
                                rhs=px.l1b[t][:, :],
                                start=True,
                                stop=True,
                                tile_position=(0, 0),
                            )

                cstate = px.c0 if layer == 0 else px.c1
                if layer == 0:
                    # merged sigmoid over [I|F|O]; biases already accumulated
                    # into PSUM by the K=3 x-projection matmuls.
                    sifo = gp.tile([P, 3 * FD], F32, name="sifo")
                    nc.scalar.activation(sifo[:], ps[:, 0 : 3 * FD], AF.Sigmoid)
                    si = sifo[:, 0:FD]
                    sf = sifo[:, FD : 2 * FD]
                    so_ = sifo[:, 2 * FD : 3 * FD]
                    tg = gp.tile([P, FD], F32, name="tg")
                    nc.scalar.activation(tg[:], ps[:, 3 * FD : 4 * FD], AF.Tanh)
                else:
                    bias = bl1
                    si_t = gp.tile([P, FD], F32, name="si")
                    nc.scalar.activation(
                        si_t[:], ps[:, 0:FD], AF.Sigmoid, bias=bias[:, 0:1]
                    )
                    si = si_t[:]
                    if t > 0:
                        sf_t = gp.tile([P, FD], F32, name="sf")
                        nc.scalar.activation(
                            sf_t[:], ps[:, FD : 2 * FD], AF.Sigmoid,
                            bias=bias[:, 1:2],
                        )
                        sf = sf_t[:]
                    so_t = gp.tile([P, FD], BF16, name="so_")
                    nc.scalar.activation(
                        so_t[:], ps[:, 2 * FD : 3 * FD], AF.Sigmoid,
                        bias=bias[:, 2:3],
                    )
                    so_ = so_t[:]
                    tg = gp.tile([P, FD], F32, name="tg1")
                    nc.scalar.activation(
                        tg[:], ps[:, 3 * FD : 4 * FD], AF.Tanh, bias=bias[:, 3:4]
                    )

                if t == 0:
                    nc.vector.tensor_mul(cstate[:], si, tg[:])
                else:
                    t1 = gp.tile([P, FD], F32, name="t1")
                    nc.vector.tensor_mul(t1[:], si, tg[:])
                    t2 = gp.tile([P, FD], F32, name="t2")
                    nc.gpsimd.tensor_mul(t2[:], sf, cstate[:])
                    nc.vector.tensor_add(cstate[:], t1[:], t2[:])

                tcx = gp.tile([P, FD], BF16, name="tcx")
                nc.scalar.activation(tcx[:], cstate[:], AF.Tanh)

                if layer == 0:
                    dlo = px.l1a[t][0:64, :]
                    dhi = px.l1b[t][64:128, :]
                elif t < 3:
                    dlo = px.l1b[t + 1][0:64, :]
                    dhi = px.l1a[t + 1][64:128, :]
                else:
                    dlo = px.mlph[0:64, :]
                    dhi = px.mlph[64:128, :]
                nc.vector.tensor_mul(dlo, so_[0:64, :], tcx[0:64, :])
                nc.vector.tensor_mul(dhi, so_[64:128, :], tcx[64:128, :])

            # ============ phase 1: LSTM stack, two pairs in lockstep ========
            for grp in range((npair + 1) // 2):
                pxs = [open_pair(q) for q in range(2 * grp, min(2 * grp + 2, npair))]
                for layer in (0, 1):
                    for t in range(4):
                        for px in pxs:
                            emit_step(px, layer, t)
                for px in pxs:
                    persist.append((px.mlph, px.ft))

            # ================= phase 2: MLP + log_softmax ===================
            for p, (mlph, ft) in enumerate(persist):
                ps = pp.tile([128, 2048], F32, name="ps")
                # MLP layer 1: A -> cols 0:512, B -> cols 512:1024
                nc.tensor.matmul(
                    ps[0:30, 0:512], lhsT=w1h[64:128, :], rhs=mlph[64:128, :],
                    start=True, stop=False, tile_position=(64, 0),
                )
                nc.tensor.matmul(
                    ps[0:30, 0:512], lhsT=w1f[32:37, :], rhs=ft[32:37, :],
                    start=False, stop=True, tile_position=(32, 0),
                )
                nc.tensor.matmul(
                    ps[0:30, 512:1024], lhsT=w1h[0:64, :], rhs=mlph[0:64, :],
                    start=True, stop=False, tile_position=(0, 0),
                )
                nc.tensor.matmul(
                    ps[0:30, 512:1024], lhsT=w1f[0:5, :], rhs=ft[0:5, :],
                    start=False, stop=True, tile_position=(0, 0),
                )
                m1s = p2.tile([30, 1024], BF16, name="m1s")
                nc.scalar.activation(
                    m1s[:], ps[0:30, 0:1024], AF.Relu, bias=bm1[:]
                )
                nc.tensor.matmul(
                    ps[0:10, 1024:1536], lhsT=wm2[:], rhs=m1s[:, 0:512],
                    start=True, stop=True, tile_position=(0, 0),
                )
                nc.tensor.matmul(
                    ps[0:10, 1536:2048], lhsT=wm2[:], rhs=m1s[:, 512:1024],
                    start=True, stop=True, tile_position=(0, 0),
                )
                m2s = p2.tile([11, 1024], BF16, name="m2s")
                nc.scalar.activation(
                    m2s[0:10, :], ps[0:10, 1024:2048], AF.Relu, bias=bm2[:]
                )
                nc.sync.dma_start(m2s[10:11, :], onesbf[:])
                nc.tensor.matmul(
                    ps[0:4, 0:512], lhsT=wm3[:], rhs=m2s[:, 0:512],
                    start=True, stop=True, tile_position=(0, 0),
                )
                nc.tensor.matmul(
                    ps[0:4, 512:1024], lhsT=wm3[:], rhs=m2s[:, 512:1024],
                    start=True, stop=True, tile_position=(0, 0),
                )
                es = p2.tile([4, 1024], F32, name="es")
                nc.scalar.activation(es[:], ps[0:4, 0:1024], AF.Exp)
                nc.tensor.matmul(
                    ps[32:33, 1024:1536], lhsT=ones4[:], rhs=es[:, 0:512],
                    start=True, stop=True, tile_position=(0, 32),
                )
                nc.tensor.matmul(
                    ps[32:33, 1536:2048], lhsT=ones4[:], rhs=es[:, 512:1024],
                    start=True, stop=True, tile_position=(0, 32),
                )
                ls = p2.tile([33, 1024], F32, name="ls")
                nc.scalar.activation(ls[32:33, :], ps[32:33, 1024:2048], AF.Ln)
                nc.tensor.matmul(
                    ps[0:4, 0:512], lhsT=nones[32:33, :], rhs=ls[32:33, 0:512],
                    start=False, stop=True, tile_position=(32, 0),
                    skip_group_check=True,
                )
                nc.tensor.matmul(
                    ps[0:4, 512:1024], lhsT=nones[32:33, :],
                    rhs=ls[32:33, 512:1024],
                    start=False, stop=True, tile_position=(32, 0),
                    skip_group_check=True,
                )
                fo = p2.tile([4, 1024], F32, name="fo")
                nc.vector.tensor_copy(fo[:], ps[0:4, 0:1024])
                nc.sync.dma_start(out_d[:, 1024 * p : 1024 * (p + 1)], fo[:])

    return nc


def pack_weights(Wih0, Whh0, bih0, bhh0, Wih1, Whh1, bih1, bhh1,
                 W1, b1, W2, b2, W3, b3):
    bf = ml_dtypes.bfloat16
    b0 = bih0 + bhh0
    b1l = bih1 + bhh1
    wh0 = np.zeros((128, 256), np.float32)
    wx0 = np.zeros((128, 256), np.float32)
    w1a = np.zeros((128, 256), np.float32)
    w1b = np.zeros((128, 256), np.float32)
    for r, sl in enumerate(GSLICE):
        gc = slice(r * 64, (r + 1) * 64)
        wh0[0:64, gc] = Whh0[sl].T
        wh0[64:128, gc] = Whh0[sl].T
        wx0[32 * r : 32 * r + 2, gc] = Wih0[sl].T
        wx0[32 * r + 2, gc] = b0[sl]
        w1a[0:64, gc] = Wih1[sl].T
        w1a[64:128, gc] = Whh1[sl].T
        w1b[0:64, gc] = Whh1[sl].T
        w1b[64:128, gc] = Wih1[sl].T
    w1h = np.zeros((128, 30), np.float32)
    w1h[0:64] = W1[:, 0:64].T
    w1h[64:128] = W1[:, 0:64].T
    w1f = np.zeros((37, 30), np.float32)
    w1f[0:5] = W1[:, 64:69].T
    w1f[32:37] = W1[:, 64:69].T
    wm2 = np.ascontiguousarray(W2.T)
    wm3 = np.zeros((11, 4), np.float32)
    wm3[0:10] = W3.T
    wm3[10] = b3
    bl1 = np.zeros((128, 4), np.float32)
    for r, sl in enumerate(GSLICE):
        bl1[0:64, r] = b1l[sl]
        bl1[64:128, r] = b1l[sl]
    return {
        "wh0": wh0.astype(bf),
        "wx0": wx0.astype(bf),
        "w1a": w1a.astype(bf),
        "w1b": w1b.astype(bf),
        "w1h": w1h.astype(bf),
        "w1f": w1f.astype(bf),
        "wm2": wm2.astype(bf),
        "wm3": wm3.astype(bf),
        "bl1": bl1,
        "bm1": np.ascontiguousarray(b1.reshape(30, 1), dtype=np.float32),
        "bm2": np.ascontiguousarray(b2.reshape(10, 1), dtype=np.float32),
        "negones": np.full((33, 4), -1.0, np.float32),
        "ones4": np.ones((4, 1), np.float32),
    }


def pack_x(xs):
    """xs: [n, 13] f32 -> xq [4, 128, n] bf16 (n multiple of FD).

    Block t: rows 32r:32r+2 = x_t (replicated per gate row-group r),
    row 32r+2 = 1.0 (bias row). Block 0 additionally carries the MLP
    feats at rows 8:13.
    """
    n = xs.shape[0]
    xqv = np.zeros((4, 128, n), np.float32)
    for t in range(4):
        for r in range(4):
            xqv[t, 32 * r] = xs[:, 2 * t]
            xqv[t, 32 * r + 1] = xs[:, 2 * t + 1]
            xqv[t, 32 * r + 2] = 1.0
    xqv[0, 8:13] = xs[:, 8:13].T
    return xqv.astype(ml_dtypes.bfloat16)


_cached = {}


def run_cores(x, weights, trace=False):
    """x: [B_TOTAL, 13] f32. Returns (out [B_TOTAL, 4] f32, BassKernelResults)."""
    key = "prog"
    if key not in _cached:
        _cached[key] = build_program(NCHUNK)
    nc = _cached[key]
    in_maps = []
    for c in range(NCORES):
        xs = x[c * B_CORE : (c + 1) * B_CORE]
        m = dict(weights)
        m["xq"] = pack_x(xs)
        in_maps.append(m)
    res = run_bass_kernel_spmd(
        nc, in_maps, core_ids=list(range(NCORES)), trace=trace
    )
    outs = [res.results[c]["out"] for c in range(NCORES)]  # [4, 16384] each
    full = np.concatenate([o.T for o in outs], axis=0)     # [B_TOTAL, 4]
    return np.ascontiguousarray(full, dtype=np.float32), res


def kernel(x, Wih0, Whh0, bih0, bhh0, Wih1, Whh1, bih1, bhh1,
           W1, b1, W2, b2, W3, b3):
    args = [np.asarray(a, dtype=np.float32) for a in (
        Wih0, Whh0, bih0, bhh0, Wih1, Whh1, bih1, bhh1, W1, b1, W2, b2, W3, b3
    )]
    weights = pack_weights(*args)
    out, _ = run_cores(np.asarray(x, dtype=np.float32), weights)
    return out


# revision 9
# speedup vs baseline: 1.6579x; 1.2856x over previous
"""Trainium2 Bass kernel for nn_ActorNetwork (2-layer LSTM [T=4,H=64] + 3-layer
MLP + log_softmax over a batch of 131072 13-dim states).

Strategy: pure data parallel over 8 NeuronCores (16384 samples/core).
On-chip layout is feature-major (gate-major): gates/hidden units live on SBUF
partitions, samples on the free axis. Two 512-sample subtiles ("A" at
partitions 0:64, "B" at 64:128) are pair-packed so ACT/DVE ops run with all
128 partitions busy; gate matmuls use block-diagonal weights so one
K=128/M=128 matmul produces the gate for both subtiles. Two pairs are
emitted in lockstep so engines pipeline across them. Matmuls are bf16 with
fp32 PSUM accumulation; the cell state c and the softmax tail stay fp32.
LSTM biases ride the matmuls via ones-rows in the x blocks (layer 0) and
per-partition ACT bias (layer 1).
"""

import numpy as np
import ml_dtypes

import concourse.bass as bass
import concourse.mybir as mybir
from concourse.tile import TileContext
from concourse.bass_utils import run_bass_kernel_spmd
from concourse.vector_clock import ScopedClock
import concourse.tile as _tile_mod

BF16 = mybir.dt.bfloat16
F32 = mybir.dt.float32
AF = mybir.ActivationFunctionType

P = 128
FD = 512          # samples per subtile (= matmul free dim = one psum bank)
H = 64
NCORES = 8
B_TOTAL = 131072
B_CORE = B_TOTAL // NCORES          # 16384
NCHUNK = B_CORE // FD               # 32 subtiles/core
NPAIR = NCHUNK // 2                 # 16 pairs/core

# psum column region r -> PyTorch gate row range (PyTorch gate order i,f,g,o)
# regions ordered [i, f, o, g] so sigmoid covers a contiguous [I|F|O] block
GSLICE = [slice(0, 64), slice(64, 128), slice(192, 256), slice(128, 192)]

# ---------------------------------------------------------------------------
# walrus workaround: this toolchain rejects instructions carrying more than
# one sync wait; split excess waits onto same-engine nops inserted right
# before the offending instruction (identical engine-stream semantics).
_WAIT_LIMIT = 1


def _split_excess_waits(nc, limit=_WAIT_LIMIT):
    for f in nc.m.functions:
        for bb in f.blocks:
            snapshot = list(bb.instructions)
            out = []
            changed = False
            for inst in snapshot:
                si = getattr(inst, "sync_info", None)
                waits = list(si.on_wait) if si is not None else []
                if len(waits) > limit:
                    changed = True
                    extra, keep = waits[:-limit], waits[-limit:]
                    for w in extra:
                        b = nc.engines[inst.engine].nop(
                            nofuse=True, hint="wsplit"
                        )
                        ni = b.ins
                        cb = nc.cur_bb.bb
                        cb.instructions.remove(ni)
                        ni.sync_info = mybir.SyncInfo(
                            on_wait=[w], on_update=[]
                        )
                        out.append(ni)
                    inst.sync_info = mybir.SyncInfo(
                        on_wait=keep, on_update=list(si.on_update)
                    )
                out.append(inst)
            if changed:
                bb.instructions[:] = out


def _patched_drain_and_barrier(self, tick_clock, wait_clock):
    nc = self.nc
    drain_inst = nc.sync.drain()
    wait_clock.add_sem_waits(
        drain_inst.ins, ScopedClock({None: tick_clock.global_clock})
    )
    si = drain_inst.ins.sync_info
    waits = list(si.on_wait) if si is not None else []
    if len(waits) > _WAIT_LIMIT:
        drain_inst.ins.sync_info = mybir.SyncInfo(
            on_wait=waits[:_WAIT_LIMIT], on_update=list(si.on_update)
        )
        for k in range(_WAIT_LIMIT, len(waits), _WAIT_LIMIT):
            d2 = nc.sync.drain()
            d2.ins.sync_info = mybir.SyncInfo(
                on_wait=waits[k : k + _WAIT_LIMIT], on_update=[]
            )
    nc.all_engine_barrier()
    popped = nc._tile_sem_poison_stack.pop()
    assert popped is self._sem_poison
    nc.clear_and_free_semaphores(list(self.sems.allocated().values()))
    nc.all_engine_barrier()
    _split_excess_waits(nc)


_tile_mod.TileContext._drain_and_barrier = _patched_drain_and_barrier
# ---------------------------------------------------------------------------


class _PairCtx:
    """SBUF tiles and per-pair state for one in-flight pair."""

    __slots__ = ("idx", "xp6", "ft", "h0", "h1", "mlph", "c0", "c1")


def build_program(nchunk=NCHUNK):
    """Build the SPMD Bass program for one core processing nchunk*FD samples."""
    assert nchunk % 2 == 0
    npair = nchunk // 2
    ncols = nchunk * FD
    pcols = npair * FD

    nc = bass.Bass("TRN2", num_devices=NCORES)

    xq = nc.declare_dram_parameter("xq", [4, 6, pcols], BF16, isOutput=False)
    fq = nc.declare_dram_parameter("fq", [5, ncols], BF16, isOutput=False)
    wx6_d = nc.declare_dram_parameter("wx6", [6, 512], BF16, isOutput=False)
    wh0_d = nc.declare_dram_parameter("wh0d", [128, 512], BF16, isOutput=False)
    w1i_d = nc.declare_dram_parameter("w1i", [128, 512], BF16, isOutput=False)
    w1r_d = nc.declare_dram_parameter("w1r", [128, 512], BF16, isOutput=False)
    w1h_d = nc.declare_dram_parameter("w1h", [128, 30], BF16, isOutput=False)
    w1f_d = nc.declare_dram_parameter("w1f", [37, 30], BF16, isOutput=False)
    wm2_d = nc.declare_dram_parameter("wm2", [30, 10], BF16, isOutput=False)
    wm3_d = nc.declare_dram_parameter("wm3", [11, 4], BF16, isOutput=False)
    bl1_d = nc.declare_dram_parameter("bl1", [128, 4], F32, isOutput=False)
    bm1_d = nc.declare_dram_parameter("bm1", [30, 1], F32, isOutput=False)
    bm2_d = nc.declare_dram_parameter("bm2", [10, 1], F32, isOutput=False)
    no_d = nc.declare_dram_parameter("negones", [33, 4], F32, isOutput=False)
    o4_d = nc.declare_dram_parameter("ones4", [4, 1], F32, isOutput=False)
    out_d = nc.declare_dram_parameter("out", [4, ncols], F32, isOutput=True)

    with TileContext(nc) as tc:
        with (
            tc.tile_pool(name="const", bufs=1) as const,
            tc.tile_pool(name="xpool", bufs=3) as xpool,
            tc.tile_pool(name="gp", bufs=3) as gp,
            tc.tile_pool(name="hp", bufs=3) as hp,
            tc.tile_pool(name="st", bufs=3) as st,
            tc.tile_pool(name="pers", bufs=npair) as pers,
            tc.tile_pool(name="pp", bufs=2, space="PSUM") as pp,
            tc.tile_pool(name="p2", bufs=2) as p2,
        ):
            # ---- constants -------------------------------------------------
            wx6 = const.tile([6, 512], BF16, name="wx6")
            nc.sync.dma_start(wx6[:], wx6_d[:, :])
            wh0 = const.tile([128, 512], BF16, name="wh0")
            nc.sync.dma_start(wh0[:], wh0_d[:, :])
            w1i = const.tile([128, 512], BF16, name="w1i")
            nc.sync.dma_start(w1i[:], w1i_d[:, :])
            w1r = const.tile([128, 512], BF16, name="w1r")
            nc.sync.dma_start(w1r[:], w1r_d[:, :])
            w1h = const.tile([128, 30], BF16, name="w1h")
            nc.sync.dma_start(w1h[:], w1h_d[:, :])
            w1f = const.tile([37, 30], BF16, name="w1f")
            nc.sync.dma_start(w1f[:], w1f_d[:, :])
            wm2 = const.tile([30, 10], BF16, name="wm2")
            nc.sync.dma_start(wm2[:], wm2_d[:, :])
            wm3 = const.tile([11, 4], BF16, name="wm3")
            nc.sync.dma_start(wm3[:], wm3_d[:, :])
            bl1 = const.tile([128, 4], F32, name="bl1")
            nc.sync.dma_start(bl1[:], bl1_d[:, :])
            bm1 = const.tile([30, 1], F32, name="bm1")
            nc.sync.dma_start(bm1[:], bm1_d[:, :])
            bm2 = const.tile([10, 1], F32, name="bm2")
            nc.sync.dma_start(bm2[:], bm2_d[:, :])
            nones = const.tile([33, 4], F32, name="nones")
            nc.sync.dma_start(nones[:], no_d[:, :])
            ones4 = const.tile([4, 1], F32, name="ones4")
            nc.sync.dma_start(ones4[:], o4_d[:, :])
            onesbf = const.tile([1, 1024], BF16, name="onesbf")
            nc.vector.memset(onesbf[:], 1.0)

            persist = []

            def open_pair(p):
                px = _PairCtx()
                px.idx = p
                pc = slice(p * FD, (p + 1) * FD)
                ca = slice(2 * p * FD, (2 * p + 1) * FD)
                cb = slice((2 * p + 1) * FD, (2 * p + 2) * FD)
                px.xp6 = []
                for t in range(4):
                    x6 = xpool.tile([6, FD], BF16, name=f"x6{t}")
                    nc.sync.dma_start(x6[:], xq[t, :, pc])
                    px.xp6.append(x6)
                ft = pers.tile([37, FD], BF16, name="ft")
                nc.sync.dma_start(ft[0:5, :], fq[:, ca])    # A feats
                nc.sync.dma_start(ft[32:37, :], fq[:, cb])  # B feats
                px.ft = ft
                px.mlph = pers.tile([P, FD], BF16, name="mlph")
                px.h0 = [hp.tile([P, FD], BF16, name=f"h0p{t}") for t in range(4)]
                # h1 state for t=0..2; t=3 goes to mlph
                px.h1 = [hp.tile([P, FD], BF16, name=f"h1p{t}") for t in range(3)]
                px.c0 = st.tile([P, FD], F32, name="c0")
                px.c1 = st.tile([P, FD], F32, name="c1")
                return px

            def emit_step(px, layer, t):
                ps = pp.tile([128, 2048], F32, name="ps")
                if layer == 0:
                    # x-projection (+bias ones-rows) first: depends only on
                    # the DMA'd x block and the psum slot, so it can run well
                    # ahead of the recurrent chain and keeps PE busy.
                    for r in range(4):
                        nc.tensor.matmul(
                            ps[:, r * FD : (r + 1) * FD],
                            lhsT=wx6[:, 128 * r : 128 * (r + 1)],
                            rhs=px.xp6[t][:, :],
                            start=True,
                            stop=(t == 0),
                            tile_position=(0, 0),
                        )
                    if t >= 1:
                        for r in range(4):
                            nc.tensor.matmul(
                                ps[:, r * FD : (r + 1) * FD],
                                lhsT=wh0[:, 128 * r : 128 * (r + 1)],
                                rhs=px.h0[t - 1][:, :],
                                start=False,
                                stop=True,
                                tile_position=(0, 0),
                            )
                else:
                    if t >= 1:
                        # recurrent part first (h1 from the previous step is
                        # the older operand -> run-ahead), then the fresh h0.
                        for r in range(4):
                            nc.tensor.matmul(
                                ps[:, r * FD : (r + 1) * FD],
                                lhsT=w1r[:, 128 * r : 128 * (r + 1)],
                                rhs=px.h1[t - 1][:, :],
                                start=True,
                                stop=False,
                                tile_position=(0, 0),
                            )
                    for r in range(4):
                        nc.tensor.matmul(
                            ps[:, r * FD : (r + 1) * FD],
                            lhsT=w1i[:, 128 * r : 128 * (r + 1)],
                            rhs=px.h0[t][:, :],
                            start=(t == 0),
                            stop=True,
                            tile_position=(0, 0),
                        )

                cstate = px.c0 if layer == 0 else px.c1
                if layer == 0:
                    # merged sigmoid over [I|F|O]; biases already accumulated
                    # into PSUM by the x-projection ones-rows.
                    sifo = gp.tile([P, 3 * FD], F32, name="sifo")
                    nc.scalar.activation(sifo[:], ps[:, 0 : 3 * FD], AF.Sigmoid)
                    si = sifo[:, 0:FD]
                    sf = sifo[:, FD : 2 * FD]
                    so_ = sifo[:, 2 * FD : 3 * FD]
                    tg = gp.tile([P, FD], F32, name="tg")
                    nc.scalar.activation(tg[:], ps[:, 3 * FD : 4 * FD], AF.Tanh)
                else:
                    si_t = gp.tile([P, FD], F32, name="si")
                    nc.scalar.activation(
                        si_t[:], ps[:, 0:FD], AF.Sigmoid, bias=bl1[:, 0:1]
                    )
                    si = si_t[:]
                    if t > 0:
                        sf_t = gp.tile([P, FD], F32, name="sf")
                        nc.scalar.activation(
                            sf_t[:], ps[:, FD : 2 * FD], AF.Sigmoid,
                            bias=bl1[:, 1:2],
                        )
                        sf = sf_t[:]
                    so_t = gp.tile([P, FD], BF16, name="so_")
                    nc.scalar.activation(
                        so_t[:], ps[:, 2 * FD : 3 * FD], AF.Sigmoid,
                        bias=bl1[:, 2:3],
                    )
                    so_ = so_t[:]
                    tg = gp.tile([P, FD], F32, name="tg1")
                    nc.scalar.activation(
                        tg[:], ps[:, 3 * FD : 4 * FD], AF.Tanh, bias=bl1[:, 3:4]
                    )

                if t == 0:
                    nc.vector.tensor_mul(cstate[:], si, tg[:])
                else:
                    t1 = gp.tile([P, FD], F32, name="t1")
                    nc.vector.tensor_mul(t1[:], si, tg[:])
                    t2 = gp.tile([P, FD], F32, name="t2")
                    nc.gpsimd.tensor_mul(t2[:], sf, cstate[:])
                    nc.vector.tensor_add(cstate[:], t1[:], t2[:])

                tcx = gp.tile([P, FD], BF16, name="tcx")
                nc.scalar.activation(tcx[:], cstate[:], AF.Tanh)

                if layer == 0:
                    dst = px.h0[t]
                elif t < 3:
                    dst = px.h1[t]
                else:
                    dst = px.mlph
                nc.vector.tensor_mul(dst[:], so_, tcx[:])

            # ============ phase 1: LSTM stack, two pairs in lockstep ========
            for grp in range((npair + 1) // 2):
                pxs = [open_pair(q) for q in range(2 * grp, min(2 * grp + 2, npair))]
                for layer in (0, 1):
                    for t in range(4):
                        for px in pxs:
                            emit_step(px, layer, t)
                for px in pxs:
                    persist.append((px.mlph, px.ft))

            # ================= phase 2: MLP + log_softmax ===================
            for p, (mlph, ft) in enumerate(persist):
                ps = pp.tile([128, 2048], F32, name="ps")
                # MLP layer 1: A -> cols 0:512, B -> cols 512:1024
                nc.tensor.matmul(
                    ps[0:30, 0:512], lhsT=w1h[0:64, :], rhs=mlph[0:64, :],
                    start=True, stop=False, tile_position=(0, 0),
                )
                nc.tensor.matmul(
                    ps[0:30, 0:512], lhsT=w1f[0:5, :], rhs=ft[0:5, :],
                    start=False, stop=True, tile_position=(0, 0),
                )
                nc.tensor.matmul(
                    ps[0:30, 512:1024], lhsT=w1h[64:128, :], rhs=mlph[64:128, :],
                    start=True, stop=False, tile_position=(64, 0),
                )
                nc.tensor.matmul(
                    ps[0:30, 512:1024], lhsT=w1f[32:37, :], rhs=ft[32:37, :],
                    start=False, stop=True, tile_position=(32, 0),
                )
                m1s = p2.tile([30, 1024], BF16, name="m1s")
                nc.scalar.activation(
                    m1s[:], ps[0:30, 0:1024], AF.Relu, bias=bm1[:]
                )
                nc.tensor.matmul(
                    ps[0:10, 1024:1536], lhsT=wm2[:], rhs=m1s[:, 0:512],
                    start=True, stop=True, tile_position=(0, 0),
                )
                nc.tensor.matmul(
                    ps[0:10, 1536:2048], lhsT=wm2[:], rhs=m1s[:, 512:1024],
                    start=True, stop=True, tile_position=(0, 0),
                )
                m2s = p2.tile([11, 1024], BF16, name="m2s")
                nc.scalar.activation(
                    m2s[0:10, :], ps[0:10, 1024:2048], AF.Relu, bias=bm2[:]
                )
                nc.sync.dma_start(m2s[10:11, :], onesbf[:])
                nc.tensor.matmul(
                    ps[0:4, 0:512], lhsT=wm3[:], rhs=m2s[:, 0:512],
                    start=True, stop=True, tile_position=(0, 0),
                )
                nc.tensor.matmul(
                    ps[0:4, 512:1024], lhsT=wm3[:], rhs=m2s[:, 512:1024],
                    start=True, stop=True, tile_position=(0, 0),
                )
                es = p2.tile([4, 1024], F32, name="es")
                nc.scalar.activation(es[:], ps[0:4, 0:1024], AF.Exp)
                nc.tensor.matmul(
                    ps[32:33, 1024:1536], lhsT=ones4[:], rhs=es[:, 0:512],
                    start=True, stop=True, tile_position=(0, 32),
                )
                nc.tensor.matmul(
                    ps[32:33, 1536:2048], lhsT=ones4[:], rhs=es[:, 512:1024],
                    start=True, stop=True, tile_position=(0, 32),
                )
                ls = p2.tile([33, 1024], F32, name="ls")
                nc.scalar.activation(ls[32:33, :], ps[32:33, 1024:2048], AF.Ln)
                nc.tensor.matmul(
                    ps[0:4, 0:512], lhsT=nones[32:33, :], rhs=ls[32:33, 0:512],
                    start=False, stop=True, tile_position=(32, 0),
                    skip_group_check=True,
                )
                nc.tensor.matmul(
                    ps[0:4, 512:1024], lhsT=nones[32:33, :],
                    rhs=ls[32:33, 512:1024],
                    start=False, stop=True, tile_position=(32, 0),
                    skip_group_check=True,
                )
                fo = p2.tile([4, 1024], F32, name="fo")
                nc.vector.tensor_copy(fo[:], ps[0:4, 0:1024])
                nc.sync.dma_start(out_d[:, 1024 * p : 1024 * (p + 1)], fo[:])

    return nc


def pack_weights(Wih0, Whh0, bih0, bhh0, Wih1, Whh1, bih1, bhh1,
                 W1, b1, W2, b2, W3, b3):
    bf = ml_dtypes.bfloat16
    b0 = bih0 + bhh0
    b1l = bih1 + bhh1
    wx6 = np.zeros((6, 512), np.float32)
    wh0 = np.zeros((128, 512), np.float32)
    w1i = np.zeros((128, 512), np.float32)
    w1r = np.zeros((128, 512), np.float32)
    for r, sl in enumerate(GSLICE):
        cA = slice(128 * r, 128 * r + 64)
        cB = slice(128 * r + 64, 128 * r + 128)
        wx6[0:2, cA] = Wih0[sl].T
        wx6[2, cA] = b0[sl]
        wx6[3:5, cB] = Wih0[sl].T
        wx6[5, cB] = b0[sl]
        wh0[0:64, cA] = Whh0[sl].T
        wh0[64:128, cB] = Whh0[sl].T
        w1i[0:64, cA] = Wih1[sl].T
        w1i[64:128, cB] = Wih1[sl].T
        w1r[0:64, cA] = Whh1[sl].T
        w1r[64:128, cB] = Whh1[sl].T
    w1h = np.zeros((128, 30), np.float32)
    w1h[0:64] = W1[:, 0:64].T
    w1h[64:128] = W1[:, 0:64].T
    w1f = np.zeros((37, 30), np.float32)
    w1f[0:5] = W1[:, 64:69].T
    w1f[32:37] = W1[:, 64:69].T
    wm2 = np.ascontiguousarray(W2.T)
    wm3 = np.zeros((11, 4), np.float32)
    wm3[0:10] = W3.T
    wm3[10] = b3
    bl1 = np.zeros((128, 4), np.float32)
    for r, sl in enumerate(GSLICE):
        bl1[0:64, r] = b1l[sl]
        bl1[64:128, r] = b1l[sl]
    return {
        "wx6": wx6.astype(bf),
        "wh0d": wh0.astype(bf),
        "w1i": w1i.astype(bf),
        "w1r": w1r.astype(bf),
        "w1h": w1h.astype(bf),
        "w1f": w1f.astype(bf),
        "wm2": wm2.astype(bf),
        "wm3": wm3.astype(bf),
        "bl1": bl1,
        "bm1": np.ascontiguousarray(b1.reshape(30, 1), dtype=np.float32),
        "bm2": np.ascontiguousarray(b2.reshape(10, 1), dtype=np.float32),
        "negones": np.full((33, 4), -1.0, np.float32),
        "ones4": np.ones((4, 1), np.float32),
    }


def pack_x(xs):
    """xs: [n, 13] f32 -> (xq [4, 6, n//2], fq [5, n]) bf16.

    xq block t column j of pair p: rows 0:2 = x_t of subtile-A sample,
    row 2 = 1.0 (bias row), rows 3:5 = x_t of subtile-B sample, row 5 = 1.0.
    fq carries the 5 MLP feats in natural chunk order.
    """
    n = xs.shape[0]
    npair = n // (2 * FD)
    a = xs.reshape(npair, 2, FD, 13)
    A = a[:, 0].reshape(npair * FD, 13)   # subtile-A samples, pair-major
    Bv = a[:, 1].reshape(npair * FD, 13)  # subtile-B samples
    xqv = np.zeros((4, 6, npair * FD), np.float32)
    for t in range(4):
        xqv[t, 0:2] = A[:, 2 * t : 2 * t + 2].T
        xqv[t, 2] = 1.0
        xqv[t, 3:5] = Bv[:, 2 * t : 2 * t + 2].T
        xqv[t, 5] = 1.0
    fqv = np.ascontiguousarray(xs[:, 8:13].T)
    return xqv.astype(ml_dtypes.bfloat16), fqv.astype(ml_dtypes.bfloat16)


_cached = {}


def run_cores(x, weights, trace=False):
    """x: [B_TOTAL, 13] f32. Returns (out [B_TOTAL, 4] f32, BassKernelResults)."""
    key = "prog"
    if key not in _cached:
        _cached[key] = build_program(NCHUNK)
    nc = _cached[key]
    in_maps = []
    for c in range(NCORES):
        xs = x[c * B_CORE : (c + 1) * B_CORE]
        m = dict(weights)
        m["xq"], m["fq"] = pack_x(xs)
        in_maps.append(m)
    res = run_bass_kernel_spmd(
        nc, in_maps, core_ids=list(range(NCORES)), trace=trace
    )
    outs = [res.results[c]["out"] for c in range(NCORES)]  # [4, 16384] each
    full = np.concatenate([o.T for o in outs], axis=0)     # [B_TOTAL, 4]
    return np.ascontiguousarray(full, dtype=np.float32), res


def kernel(x, Wih0, Whh0, bih0, bhh0, Wih1, Whh1, bih1, bhh1,
           W1, b1, W2, b2, W3, b3):
    args = [np.asarray(a, dtype=np.float32) for a in (
        Wih0, Whh0, bih0, bhh0, Wih1, Whh1, bih1, bhh1, W1, b1, W2, b2, W3, b3
    )]
    weights = pack_weights(*args)
    out, _ = run_cores(np.asarray(x, dtype=np.float32), weights)
    return out


# revision 10
# speedup vs baseline: 1.7359x; 1.0470x over previous
"""Trainium2 Bass kernel for nn_ActorNetwork (2-layer LSTM [T=4,H=64] + 3-layer
MLP + log_softmax over a batch of 131072 13-dim states).

Strategy: pure data parallel over 8 NeuronCores (16384 samples/core).
On-chip layout is feature-major (gate-major): gates/hidden units live on SBUF
partitions, samples on the free axis. Two 512-sample subtiles ("A" at
partitions 0:64, "B" at 64:128) are pair-packed so ACT/DVE ops run with all
128 partitions busy; gate matmuls use block-diagonal weights so one
K=128/M=128 matmul produces the gate for both subtiles. Two pairs are
emitted in lockstep so engines pipeline across them. Matmuls are bf16 with
fp32 PSUM accumulation; the cell state c and the softmax tail stay fp32.
All LSTM biases ride matmuls (ones-rows in the x blocks for layer 0, a
dedicated K=6 const matmul for layer 1) so the sigmoid over [I|F|O] is one
merged ACT op per step.
"""

import numpy as np
import ml_dtypes

import concourse.bass as bass
import concourse.mybir as mybir
from concourse.tile import TileContext
from concourse.bass_utils import run_bass_kernel_spmd
from concourse.vector_clock import ScopedClock
import concourse.tile as _tile_mod

BF16 = mybir.dt.bfloat16
F32 = mybir.dt.float32
AF = mybir.ActivationFunctionType

P = 128
FD = 512          # samples per subtile (= matmul free dim = one psum bank)
H = 64
NCORES = 8
B_TOTAL = 131072
B_CORE = B_TOTAL // NCORES          # 16384
NCHUNK = B_CORE // FD               # 32 subtiles/core
NPAIR = NCHUNK // 2                 # 16 pairs/core

# psum column region r -> PyTorch gate row range (PyTorch gate order i,f,g,o)
# regions ordered [i, f, o, g] so sigmoid covers a contiguous [I|F|O] block
GSLICE = [slice(0, 64), slice(64, 128), slice(192, 256), slice(128, 192)]
RORD = (3, 0, 1, 2)  # emit G first so tanh(G) unblocks the c-chain earliest

# ---------------------------------------------------------------------------
# walrus workaround: this toolchain rejects instructions carrying more than
# one sync wait; split excess waits onto same-engine nops inserted right
# before the offending instruction (identical engine-stream semantics).
_WAIT_LIMIT = 1


def _split_excess_waits(nc, limit=_WAIT_LIMIT):
    for f in nc.m.functions:
        for bb in f.blocks:
            snapshot = list(bb.instructions)
            out = []
            changed = False
            for inst in snapshot:
                si = getattr(inst, "sync_info", None)
                waits = list(si.on_wait) if si is not None else []
                if len(waits) > limit:
                    changed = True
                    extra, keep = waits[:-limit], waits[-limit:]
                    for w in extra:
                        b = nc.engines[inst.engine].nop(
                            nofuse=True, hint="wsplit"
                        )
                        ni = b.ins
                        cb = nc.cur_bb.bb
                        cb.instructions.remove(ni)
                        ni.sync_info = mybir.SyncInfo(
                            on_wait=[w], on_update=[]
                        )
                        out.append(ni)
                    inst.sync_info = mybir.SyncInfo(
                        on_wait=keep, on_update=list(si.on_update)
                    )
                out.append(inst)
            if changed:
                bb.instructions[:] = out


def _patched_drain_and_barrier(self, tick_clock, wait_clock):
    nc = self.nc
    drain_inst = nc.sync.drain()
    wait_clock.add_sem_waits(
        drain_inst.ins, ScopedClock({None: tick_clock.global_clock})
    )
    si = drain_inst.ins.sync_info
    waits = list(si.on_wait) if si is not None else []
    if len(waits) > _WAIT_LIMIT:
        drain_inst.ins.sync_info = mybir.SyncInfo(
            on_wait=waits[:_WAIT_LIMIT], on_update=list(si.on_update)
        )
        for k in range(_WAIT_LIMIT, len(waits), _WAIT_LIMIT):
            d2 = nc.sync.drain()
            d2.ins.sync_info = mybir.SyncInfo(
                on_wait=waits[k : k + _WAIT_LIMIT], on_update=[]
            )
    nc.all_engine_barrier()
    popped = nc._tile_sem_poison_stack.pop()
    assert popped is self._sem_poison
    nc.clear_and_free_semaphores(list(self.sems.allocated().values()))
    nc.all_engine_barrier()
    _split_excess_waits(nc)


_tile_mod.TileContext._drain_and_barrier = _patched_drain_and_barrier
# ---------------------------------------------------------------------------


class _PairCtx:
    __slots__ = ("idx", "xp6", "ft", "h0", "h1", "mlph", "c0", "c1")


def build_program(nchunk=NCHUNK):
    """Build the SPMD Bass program for one core processing nchunk*FD samples."""
    assert nchunk % 2 == 0
    npair = nchunk // 2
    ncols = nchunk * FD
    pcols = npair * FD

    nc = bass.Bass("TRN2", num_devices=NCORES)

    xq = nc.declare_dram_parameter("xq", [4, 6, pcols], BF16, isOutput=False)
    fq = nc.declare_dram_parameter("fq", [5, ncols], BF16, isOutput=False)
    wx6_d = nc.declare_dram_parameter("wx6", [6, 512], BF16, isOutput=False)
    wb1_d = nc.declare_dram_parameter("wb1", [6, 512], BF16, isOutput=False)
    o6_d = nc.declare_dram_parameter("ones6", [6, FD], BF16, isOutput=False)
    wh0_d = nc.declare_dram_parameter("wh0d", [128, 512], BF16, isOutput=False)
    w1i_d = nc.declare_dram_parameter("w1i", [128, 512], BF16, isOutput=False)
    w1r_d = nc.declare_dram_parameter("w1r", [128, 512], BF16, isOutput=False)
    w1h_d = nc.declare_dram_parameter("w1h", [128, 30], BF16, isOutput=False)
    w1f_d = nc.declare_dram_parameter("w1f", [37, 30], BF16, isOutput=False)
    wm2_d = nc.declare_dram_parameter("wm2", [30, 10], BF16, isOutput=False)
    wm3_d = nc.declare_dram_parameter("wm3", [11, 4], BF16, isOutput=False)
    bm1_d = nc.declare_dram_parameter("bm1", [30, 1], F32, isOutput=False)
    bm2_d = nc.declare_dram_parameter("bm2", [10, 1], F32, isOutput=False)
    no_d = nc.declare_dram_parameter("negones", [33, 4], F32, isOutput=False)
    o4_d = nc.declare_dram_parameter("ones4", [4, 1], F32, isOutput=False)
    out_d = nc.declare_dram_parameter("out", [4, ncols], F32, isOutput=True)

    with TileContext(nc) as tc:
        with (
            tc.tile_pool(name="const", bufs=1) as const,
            tc.tile_pool(name="xpool", bufs=4) as xpool,
            tc.tile_pool(name="gp", bufs=3) as gp,
            tc.tile_pool(name="hp", bufs=3) as hp,
            tc.tile_pool(name="st", bufs=3) as st,
            tc.tile_pool(name="pers", bufs=npair) as pers,
            tc.tile_pool(name="pp", bufs=2, space="PSUM") as pp,
            tc.tile_pool(name="p2", bufs=2) as p2,
        ):
            # ---- constants -------------------------------------------------
            def cdma(name, dram, shape, dt=BF16):
                t = const.tile(shape, dt, name=name)
                nc.sync.dma_start(t[:], dram[:, :])
                return t

            wx6 = cdma("wx6", wx6_d, [6, 512])
            wb1 = cdma("wb1", wb1_d, [6, 512])
            ones6 = cdma("ones6", o6_d, [6, FD])
            wh0 = cdma("wh0", wh0_d, [128, 512])
            w1i = cdma("w1i", w1i_d, [128, 512])
            w1r = cdma("w1r", w1r_d, [128, 512])
            w1h = cdma("w1h", w1h_d, [128, 30])
            w1f = cdma("w1f", w1f_d, [37, 30])
            wm2 = cdma("wm2", wm2_d, [30, 10])
            wm3 = cdma("wm3", wm3_d, [11, 4])
            bm1 = cdma("bm1", bm1_d, [30, 1], F32)
            bm2 = cdma("bm2", bm2_d, [10, 1], F32)
            nones = cdma("nones", no_d, [33, 4], F32)
            ones4 = cdma("ones4", o4_d, [4, 1], F32)
            onesbf = const.tile([1, 2048], BF16, name="onesbf")
            nc.vector.memset(onesbf[:], 1.0)

            persist = []

            def open_pair(p):
                px = _PairCtx()
                px.idx = p
                pc = slice(p * FD, (p + 1) * FD)
                ca = slice(2 * p * FD, (2 * p + 1) * FD)
                cb = slice((2 * p + 1) * FD, (2 * p + 2) * FD)
                px.xp6 = []
                for t in range(4):
                    x6 = xpool.tile([6, FD], BF16, name=f"x6{t}")
                    nc.sync.dma_start(x6[:], xq[t, :, pc])
                    px.xp6.append(x6)
                ft = pers.tile([37, FD], BF16, name="ft")
                nc.sync.dma_start(ft[0:5, :], fq[:, ca])    # A feats
                nc.sync.dma_start(ft[32:37, :], fq[:, cb])  # B feats
                px.ft = ft
                px.mlph = pers.tile([P, FD], BF16, name="mlph")
                px.h0 = [hp.tile([P, FD], BF16, name=f"h0p{t}") for t in range(4)]
                px.h1 = [hp.tile([P, FD], BF16, name=f"h1p{t}") for t in range(3)]
                px.c0 = st.tile([P, FD], F32, name="c0")
                px.c1 = st.tile([P, FD], F32, name="c1")
                return px

            def emit_step(px, layer, t):
                ps = pp.tile([128, 2048], F32, name="ps")
                if layer == 0:
                    # x-projection (+bias ones-rows): depends only on the
                    # DMA'd x block and the psum slot -> runs well ahead of
                    # the recurrent chain and keeps PE busy.
                    for r in RORD:
                        nc.tensor.matmul(
                            ps[:, r * FD : (r + 1) * FD],
                            lhsT=wx6[:, 128 * r : 128 * (r + 1)],
                            rhs=px.xp6[t][:, :],
                            start=True,
                            stop=(t == 0),
                            tile_position=(0, 0),
                        )
                    if t >= 1:
                        for r in RORD:
                            nc.tensor.matmul(
                                ps[:, r * FD : (r + 1) * FD],
                                lhsT=wh0[:, 128 * r : 128 * (r + 1)],
                                rhs=px.h0[t - 1][:, :],
                                start=False,
                                stop=True,
                                tile_position=(0, 0),
                            )
                else:
                    # bias matmul from constants: maximal run-ahead
                    for r in RORD:
                        nc.tensor.matmul(
                            ps[:, r * FD : (r + 1) * FD],
                            lhsT=wb1[:, 128 * r : 128 * (r + 1)],
                            rhs=ones6[:, :],
                            start=True,
                            stop=False,
                            tile_position=(0, 0),
                        )
                    if t >= 1:
                        for r in RORD:
                            nc.tensor.matmul(
                                ps[:, r * FD : (r + 1) * FD],
                                lhsT=w1r[:, 128 * r : 128 * (r + 1)],
                                rhs=px.h1[t - 1][:, :],
                                start=False,
                                stop=False,
                                tile_position=(0, 0),
                            )
                    for r in RORD:
                        nc.tensor.matmul(
                            ps[:, r * FD : (r + 1) * FD],
                            lhsT=w1i[:, 128 * r : 128 * (r + 1)],
                            rhs=px.h0[t][:, :],
                            start=False,
                            stop=True,
                            tile_position=(0, 0),
                        )

                cstate = px.c0 if layer == 0 else px.c1
                tg = gp.tile([P, FD], F32, name="tg")
                nc.scalar.activation(tg[:], ps[:, 3 * FD : 4 * FD], AF.Tanh)
                sifo = gp.tile([P, 3 * FD], F32, name="sifo")
                nc.scalar.activation(sifo[:], ps[:, 0 : 3 * FD], AF.Sigmoid)
                si = sifo[:, 0:FD]
                sf = sifo[:, FD : 2 * FD]
                so_ = sifo[:, 2 * FD : 3 * FD]

                if t == 0:
                    nc.vector.tensor_mul(cstate[:], si, tg[:])
                else:
                    t1 = gp.tile([P, FD], F32, name="t1")
                    nc.vector.tensor_mul(t1[:], si, tg[:])
                    t2 = gp.tile([P, FD], F32, name="t2")
                    nc.gpsimd.tensor_mul(t2[:], sf, cstate[:])
                    nc.vector.tensor_add(cstate[:], t1[:], t2[:])

                tcx = gp.tile([P, FD], BF16, name="tcx")
                nc.scalar.activation(tcx[:], cstate[:], AF.Tanh)

                if layer == 0:
                    dst = px.h0[t]
                elif t < 3:
                    dst = px.h1[t]
                else:
                    dst = px.mlph
                nc.vector.tensor_mul(dst[:], so_, tcx[:])

            # ============ phase 1: LSTM stack, two pairs in lockstep ========
            for grp in range((npair + 1) // 2):
                pxs = [open_pair(q) for q in range(2 * grp, min(2 * grp + 2, npair))]
                for layer in (0, 1):
                    for t in range(4):
                        for px in pxs:
                            emit_step(px, layer, t)
                for px in pxs:
                    persist.append((px.mlph, px.ft))

            # ======== phase 2: MLP + log_softmax, two pairs per block =======
            def emit_phase2(block, p0):
                nb = len(block)
                W = 1024 * nb
                ps = pp.tile([128, 2048], F32, name="ps")
                for i, (mlph, ft) in enumerate(block):
                    for half in (0, 1):
                        c = slice(1024 * i + 512 * half, 1024 * i + 512 * half + 512)
                        rows = slice(0, 64) if half == 0 else slice(64, 128)
                        frows = slice(0, 5) if half == 0 else slice(32, 37)
                        nc.tensor.matmul(
                            ps[0:30, c], lhsT=w1h[rows, :], rhs=mlph[rows, :],
                            start=True, stop=False,
                            tile_position=(0 if half == 0 else 64, 0),
                        )
                        nc.tensor.matmul(
                            ps[0:30, c], lhsT=w1f[frows, :], rhs=ft[frows, :],
                            start=False, stop=True,
                            tile_position=(0 if half == 0 else 32, 0),
                        )
                m1s = p2.tile([30, 2048], BF16, name="m1s")
                nc.scalar.activation(
                    m1s[:, 0:W], ps[0:30, 0:W], AF.Relu, bias=bm1[:]
                )
                for j in range(2 * nb):
                    c = slice(512 * j, 512 * (j + 1))
                    nc.tensor.matmul(
                        ps[0:10, c], lhsT=wm2[:], rhs=m1s[:, c],
                        start=True, stop=True, tile_position=(0, 0),
                    )
                m2s = p2.tile([11, 2048], BF16, name="m2s")
                nc.scalar.activation(
                    m2s[0:10, 0:W], ps[0:10, 0:W], AF.Relu, bias=bm2[:]
                )
                nc.sync.dma_start(m2s[10:11, 0:W], onesbf[:, 0:W])
                for j in range(2 * nb):
                    c = slice(512 * j, 512 * (j + 1))
                    nc.tensor.matmul(
                        ps[0:4, c], lhsT=wm3[:], rhs=m2s[:, c],
                        start=True, stop=True, tile_position=(0, 0),
                    )
                es = p2.tile([4, 2048], F32, name="es", bufs=2)
                nc.scalar.activation(es[:, 0:W], ps[0:4, 0:W], AF.Exp)
                for j in range(2 * nb):
                    c = slice(512 * j, 512 * (j + 1))
                    nc.tensor.matmul(
                        ps[32:33, c], lhsT=ones4[:], rhs=es[:, c],
                        start=True, stop=True, tile_position=(0, 32),
                    )
                ls = p2.tile([33, 2048], F32, name="ls", bufs=2)
                nc.scalar.activation(ls[32:33, 0:W], ps[32:33, 0:W], AF.Ln)
                for j in range(2 * nb):
                    c = slice(512 * j, 512 * (j + 1))
                    nc.tensor.matmul(
                        ps[0:4, c], lhsT=nones[32:33, :], rhs=ls[32:33, c],
                        start=False, stop=True, tile_position=(32, 0),
                        skip_group_check=True,
                    )
                fo = p2.tile([4, 2048], F32, name="fo", bufs=2)
                nc.vector.tensor_copy(fo[:, 0:W], ps[0:4, 0:W])
                nc.sync.dma_start(
                    out_d[:, 1024 * p0 : 1024 * p0 + W], fo[:, 0:W]
                )

            for b0 in range(0, npair, 2):
                emit_phase2(persist[b0 : b0 + 2], b0)

    return nc


def pack_weights(Wih0, Whh0, bih0, bhh0, Wih1, Whh1, bih1, bhh1,
                 W1, b1, W2, b2, W3, b3):
    bf = ml_dtypes.bfloat16
    b0 = bih0 + bhh0
    b1l = bih1 + bhh1
    wx6 = np.zeros((6, 512), np.float32)
    wb1 = np.zeros((6, 512), np.float32)
    wh0 = np.zeros((128, 512), np.float32)
    w1i = np.zeros((128, 512), np.float32)
    w1r = np.zeros((128, 512), np.float32)
    for r, sl in enumerate(GSLICE):
        cA = slice(128 * r, 128 * r + 64)
        cB = slice(128 * r + 64, 128 * r + 128)
        wx6[0:2, cA] = Wih0[sl].T
        wx6[2, cA] = b0[sl]
        wx6[3:5, cB] = Wih0[sl].T
        wx6[5, cB] = b0[sl]
        wb1[2, cA] = b1l[sl]
        wb1[5, cB] = b1l[sl]
        wh0[0:64, cA] = Whh0[sl].T
        wh0[64:128, cB] = Whh0[sl].T
        w1i[0:64, cA] = Wih1[sl].T
        w1i[64:128, cB] = Wih1[sl].T
        w1r[0:64, cA] = Whh1[sl].T
        w1r[64:128, cB] = Whh1[sl].T
    ones6 = np.zeros((6, FD), np.float32)
    ones6[2] = 1.0
    ones6[5] = 1.0
    w1h = np.zeros((128, 30), np.float32)
    w1h[0:64] = W1[:, 0:64].T
    w1h[64:128] = W1[:, 0:64].T
    w1f = np.zeros((37, 30), np.float32)
    w1f[0:5] = W1[:, 64:69].T
    w1f[32:37] = W1[:, 64:69].T
    wm2 = np.ascontiguousarray(W2.T)
    wm3 = np.zeros((11, 4), np.float32)
    wm3[0:10] = W3.T
    wm3[10] = b3
    return {
        "wx6": wx6.astype(bf),
        "wb1": wb1.astype(bf),
        "ones6": ones6.astype(bf),
        "wh0d": wh0.astype(bf),
        "w1i": w1i.astype(bf),
        "w1r": w1r.astype(bf),
        "w1h": w1h.astype(bf),
        "w1f": w1f.astype(bf),
        "wm2": wm2.astype(bf),
        "wm3": wm3.astype(bf),
        "bm1": np.ascontiguousarray(b1.reshape(30, 1), dtype=np.float32),
        "bm2": np.ascontiguousarray(b2.reshape(10, 1), dtype=np.float32),
        "negones": np.full((33, 4), -1.0, np.float32),
        "ones4": np.ones((4, 1), np.float32),
    }


def pack_x(xs):
    """xs: [n, 13] f32 -> (xq [4, 6, n//2], fq [5, n]) bf16."""
    n = xs.shape[0]
    npair = n // (2 * FD)
    a = xs.reshape(npair, 2, FD, 13)
    A = a[:, 0].reshape(npair * FD, 13)
    Bv = a[:, 1].reshape(npair * FD, 13)
    xqv = np.zeros((4, 6, npair * FD), np.float32)
    for t in range(4):
        xqv[t, 0:2] = A[:, 2 * t : 2 * t + 2].T
        xqv[t, 2] = 1.0
        xqv[t, 3:5] = Bv[:, 2 * t : 2 * t + 2].T
        xqv[t, 5] = 1.0
    fqv = np.ascontiguousarray(xs[:, 8:13].T)
    return xqv.astype(ml_dtypes.bfloat16), fqv.astype(ml_dtypes.bfloat16)


_cached = {}


def run_cores(x, weights, trace=False):
    """x: [B_TOTAL, 13] f32. Returns (out [B_TOTAL, 4] f32, BassKernelResults)."""
    key = "prog"
    if key not in _cached:
        _cached[key] = build_program(NCHUNK)
    nc = _cached[key]
    in_maps = []
    for c in range(NCORES):
        xs = x[c * B_CORE : (c + 1) * B_CORE]
        m = dict(weights)
        m["xq"], m["fq"] = pack_x(xs)
        in_maps.append(m)
    res = run_bass_kernel_spmd(
        nc, in_maps, core_ids=list(range(NCORES)), trace=trace
    )
    outs = [res.results[c]["out"] for c in range(NCORES)]  # [4, 16384] each
    full = np.concatenate([o.T for o in outs], axis=0)     # [B_TOTAL, 4]
    return np.ascontiguousarray(full, dtype=np.float32), res


def kernel(x, Wih0, Whh0, bih0, bhh0, Wih1, Whh1, bih1, bhh1,
           W1, b1, W2, b2, W3, b3):
    args = [np.asarray(a, dtype=np.float32) for a in (
        Wih0, Whh0, bih0, bhh0, Wih1, Whh1, bih1, bhh1, W1, b1, W2, b2, W3, b3
    )]
    weights = pack_weights(*args)
    out, _ = run_cores(np.asarray(x, dtype=np.float32), weights)
    return out


# revision 17
# speedup vs baseline: 1.8466x; 1.0638x over previous
"""Trainium2 Bass kernel for nn_ActorNetwork (2-layer LSTM [T=4,H=64] + 3-layer
MLP + log_softmax over a batch of 131072 13-dim states).

Strategy: pure data parallel over 8 NeuronCores (16384 samples/core).
On-chip layout is feature-major (gate-major): gates/hidden units live on SBUF
partitions, samples on the free axis. Two 512-sample subtiles ("A" at
partitions 0:64, "B" at 64:128) are pair-packed so ACT/DVE ops run with all
128 partitions busy; gate matmuls use block-diagonal weights so one
K=128/M=128 matmul produces the gate for both subtiles. Two pairs are
emitted in lockstep so engines pipeline across them. Matmuls are bf16 with
fp32 PSUM accumulation; the cell state c and the softmax tail stay fp32.
All LSTM biases ride matmuls (ones-rows in the x blocks for layer 0, a
dedicated K=6 const matmul for layer 1) so the sigmoid over [I|F|O] is one
merged ACT op per step.
"""

import numpy as np
import ml_dtypes

import concourse.bass as bass
import concourse.mybir as mybir
from concourse.tile import TileContext
from concourse.bass_utils import run_bass_kernel_spmd
from concourse.vector_clock import ScopedClock
import concourse.tile as _tile_mod

BF16 = mybir.dt.bfloat16
F32 = mybir.dt.float32
AF = mybir.ActivationFunctionType

P = 128
FD = 512          # samples per subtile (= matmul free dim = one psum bank)
H = 64
NCORES = 8
B_TOTAL = 131072
B_CORE = B_TOTAL // NCORES          # 16384
NCHUNK = B_CORE // FD               # 32 subtiles/core
NPAIR = NCHUNK // 2                 # 16 pairs/core

# psum column region r -> PyTorch gate row range (PyTorch gate order i,f,g,o)
# regions ordered [i, f, o, g] so sigmoid covers a contiguous [I|F|O] block
GSLICE = [slice(0, 64), slice(64, 128), slice(192, 256), slice(128, 192)]
RORD = (3, 0, 1, 2)  # emit G first so tanh(G) unblocks the c-chain earliest

# ---------------------------------------------------------------------------
# walrus workaround: this toolchain rejects instructions carrying more than
# one sync wait; split excess waits onto same-engine nops inserted right
# before the offending instruction (identical engine-stream semantics).
_WAIT_LIMIT = 1


def _split_excess_waits(nc, limit=_WAIT_LIMIT):
    for f in nc.m.functions:
        for bb in f.blocks:
            snapshot = list(bb.instructions)
            out = []
            changed = False
            for inst in snapshot:
                si = getattr(inst, "sync_info", None)
                waits = list(si.on_wait) if si is not None else []
                if len(waits) > limit:
                    changed = True
                    extra, keep = waits[:-limit], waits[-limit:]
                    for w in extra:
                        b = nc.engines[inst.engine].nop(
                            nofuse=True, hint="wsplit"
                        )
                        ni = b.ins
                        cb = nc.cur_bb.bb
                        cb.instructions.remove(ni)
                        ni.sync_info = mybir.SyncInfo(
                            on_wait=[w], on_update=[]
                        )
                        out.append(ni)
                    inst.sync_info = mybir.SyncInfo(
                        on_wait=keep, on_update=list(si.on_update)
                    )
                out.append(inst)
            if changed:
                bb.instructions[:] = out


def _patched_drain_and_barrier(self, tick_clock, wait_clock):
    nc = self.nc
    drain_inst = nc.sync.drain()
    wait_clock.add_sem_waits(
        drain_inst.ins, ScopedClock({None: tick_clock.global_clock})
    )
    si = drain_inst.ins.sync_info
    waits = list(si.on_wait) if si is not None else []
    if len(waits) > _WAIT_LIMIT:
        drain_inst.ins.sync_info = mybir.SyncInfo(
            on_wait=waits[:_WAIT_LIMIT], on_update=list(si.on_update)
        )
        for k in range(_WAIT_LIMIT, len(waits), _WAIT_LIMIT):
            d2 = nc.sync.drain()
            d2.ins.sync_info = mybir.SyncInfo(
                on_wait=waits[k : k + _WAIT_LIMIT], on_update=[]
            )
    nc.all_engine_barrier()
    popped = nc._tile_sem_poison_stack.pop()
    assert popped is self._sem_poison
    nc.clear_and_free_semaphores(list(self.sems.allocated().values()))
    nc.all_engine_barrier()
    _split_excess_waits(nc)


_tile_mod.TileContext._drain_and_barrier = _patched_drain_and_barrier
# ---------------------------------------------------------------------------


class _PairCtx:
    __slots__ = ("idx", "xp6", "ft", "h0", "h1", "mlph", "c0", "c1")


def build_program(nchunk=NCHUNK):
    """Build the SPMD Bass program for one core processing nchunk*FD samples."""
    assert nchunk % 2 == 0
    npair = nchunk // 2
    ncols = nchunk * FD
    pcols = npair * FD

    nc = bass.Bass("TRN2", num_devices=NCORES)

    xq = nc.declare_dram_parameter("xq", [4, 6, pcols], BF16, isOutput=False)
    fq = nc.declare_dram_parameter("fq", [5, ncols], BF16, isOutput=False)
    wx6_d = nc.declare_dram_parameter("wx6", [6, 512], BF16, isOutput=False)
    wh0_d = nc.declare_dram_parameter("wh0d", [128, 512], BF16, isOutput=False)
    w1i_d = nc.declare_dram_parameter("w1i", [128, 512], BF16, isOutput=False)
    w1r_d = nc.declare_dram_parameter("w1r", [128, 512], BF16, isOutput=False)
    w1h_d = nc.declare_dram_parameter("w1h", [128, 30], BF16, isOutput=False)
    w1f_d = nc.declare_dram_parameter("w1f", [37, 30], BF16, isOutput=False)
    wm2_d = nc.declare_dram_parameter("wm2", [30, 10], BF16, isOutput=False)
    wm3_d = nc.declare_dram_parameter("wm3", [43, 4], BF16, isOutput=False)
    bl1_d = nc.declare_dram_parameter("bl1", [128, 4], F32, isOutput=False)
    bm1_d = nc.declare_dram_parameter("bm1", [30, 1], F32, isOutput=False)
    bm2_d = nc.declare_dram_parameter("bm2", [42, 1], F32, isOutput=False)
    no_d = nc.declare_dram_parameter("negones", [97, 4], F32, isOutput=False)
    o4_d = nc.declare_dram_parameter("ones4", [68, 1], F32, isOutput=False)
    out_d = nc.declare_dram_parameter("out", [4, ncols], F32, isOutput=True)
    warm_d = nc.declare_dram_parameter("warm", [1, 4], F32, isOutput=True)

    with TileContext(nc) as tc:
        with (
            tc.tile_pool(name="const", bufs=1) as const,
            tc.tile_pool(name="xpool", bufs=4) as xpool,
            tc.tile_pool(name="gp", bufs=3) as gp,
            tc.tile_pool(name="hp", bufs=3) as hp,
            tc.tile_pool(name="st", bufs=3) as st,
            tc.tile_pool(name="pers", bufs=npair) as pers,
            tc.tile_pool(name="pp", bufs=2, space="PSUM") as pp,
            tc.tile_pool(name="p2", bufs=2) as p2,
        ):
            # ---- constants -------------------------------------------------
            def cdma(name, dram, shape, dt=BF16):
                t = const.tile(shape, dt, name=name)
                nc.sync.dma_start(t[:], dram[:, :])
                return t

            wx6 = cdma("wx6", wx6_d, [6, 512])
            wh0 = cdma("wh0", wh0_d, [128, 512])
            w1i = cdma("w1i", w1i_d, [128, 512])
            w1r = cdma("w1r", w1r_d, [128, 512])
            w1h = cdma("w1h", w1h_d, [128, 30])
            w1f = cdma("w1f", w1f_d, [37, 30])
            wm2 = cdma("wm2", wm2_d, [30, 10])
            wm3 = cdma("wm3", wm3_d, [43, 4])
            bl1 = cdma("bl1", bl1_d, [128, 4], F32)
            bm1 = cdma("bm1", bm1_d, [30, 1], F32)
            bm2 = cdma("bm2", bm2_d, [42, 1], F32)
            nones = cdma("nones", no_d, [97, 4], F32)
            ones4 = cdma("ones4", o4_d, [68, 1], F32)
            onesbf = const.tile([1, 2048], BF16, name="onesbf")
            nc.vector.memset(onesbf[:], 1.0)

            # ---- PE warm-up: a dense burst of back-to-back matmuls so the
            # HAM clock gate reaches K=8/8 (2.4 GHz) before the real work.
            # Result is exported to a dummy output so nothing can DCE it.
            wps = pp.tile([128, 2048], F32, name="ps")
            for k in range(24):
                bank = k % 4
                nc.tensor.matmul(
                    wps[:, bank * FD : (bank + 1) * FD],
                    lhsT=wh0[:, 0:128],
                    rhs=wh0[:, 0:512],
                    start=(k < 4),
                    stop=(k >= 20),
                    tile_position=(0, 0),
                )
            wsb = const.tile([1, 4], F32, name="wsb")
            nc.vector.tensor_copy(wsb[:], wps[0:1, 0:4])
            nc.sync.dma_start(warm_d[:, :], wsb[:])

            persist = []

            def open_pair(p):
                px = _PairCtx()
                px.idx = p
                pc = slice(p * FD, (p + 1) * FD)
                ca = slice(2 * p * FD, (2 * p + 1) * FD)
                cb = slice((2 * p + 1) * FD, (2 * p + 2) * FD)
                px.xp6 = []
                for t in range(4):
                    x6 = xpool.tile([6, FD], BF16, name=f"x6{t}")
                    nc.sync.dma_start(x6[:], xq[t, :, pc])
                    px.xp6.append(x6)
                ft = pers.tile([37, FD], BF16, name="ft")
                nc.sync.dma_start(ft[0:5, :], fq[:, ca])    # A feats
                nc.sync.dma_start(ft[32:37, :], fq[:, cb])  # B feats
                px.ft = ft
                px.mlph = pers.tile([P, FD], BF16, name="mlph")
                px.h0 = [hp.tile([P, FD], BF16, name=f"h0p{t}") for t in range(4)]
                px.h1 = [hp.tile([P, FD], BF16, name=f"h1p{t}") for t in range(3)]
                px.c0 = st.tile([P, FD], F32, name="c0")
                px.c1 = st.tile([P, FD], F32, name="c1")
                return px

            def emit_step(px, layer, t):
                ps = pp.tile([128, 2048], F32, name="ps")
                if layer == 0:
                    # x-projection (+bias ones-rows): depends only on the
                    # DMA'd x block and the psum slot -> runs well ahead of
                    # the recurrent chain and keeps PE busy.
                    for r in RORD:
                        nc.tensor.matmul(
                            ps[:, r * FD : (r + 1) * FD],
                            lhsT=wx6[:, 128 * r : 128 * (r + 1)],
                            rhs=px.xp6[t][:, :],
                            start=True,
                            stop=(t == 0),
                            tile_position=(0, 0),
                        )
                    if t >= 1:
                        for r in RORD:
                            nc.tensor.matmul(
                                ps[:, r * FD : (r + 1) * FD],
                                lhsT=wh0[:, 128 * r : 128 * (r + 1)],
                                rhs=px.h0[t - 1][:, :],
                                start=False,
                                stop=True,
                                tile_position=(0, 0),
                            )
                else:
                    if t >= 1:
                        for r in RORD:
                            nc.tensor.matmul(
                                ps[:, r * FD : (r + 1) * FD],
                                lhsT=w1r[:, 128 * r : 128 * (r + 1)],
                                rhs=px.h1[t - 1][:, :],
                                start=True,
                                stop=False,
                                tile_position=(0, 0),
                            )
                    for r in RORD:
                        nc.tensor.matmul(
                            ps[:, r * FD : (r + 1) * FD],
                            lhsT=w1i[:, 128 * r : 128 * (r + 1)],
                            rhs=px.h0[t][:, :],
                            start=(t == 0),
                            stop=True,
                            tile_position=(0, 0),
                        )

                cstate = px.c0 if layer == 0 else px.c1
                if layer == 0:
                    tg = gp.tile([P, FD], F32, name="tg")
                    nc.scalar.activation(tg[:], ps[:, 3 * FD : 4 * FD], AF.Tanh)
                    sifo = gp.tile([P, 3 * FD], F32, name="sifo")
                    nc.scalar.activation(sifo[:], ps[:, 0 : 3 * FD], AF.Sigmoid)
                    si = sifo[:, 0:FD]
                    sf = sifo[:, FD : 2 * FD]
                    so_ = sifo[:, 2 * FD : 3 * FD]
                else:
                    tg = gp.tile([P, FD], F32, name="tg")
                    nc.scalar.activation(
                        tg[:], ps[:, 3 * FD : 4 * FD], AF.Tanh, bias=bl1[:, 3:4]
                    )
                    si_t = gp.tile([P, FD], F32, name="si")
                    nc.scalar.activation(
                        si_t[:], ps[:, 0:FD], AF.Sigmoid, bias=bl1[:, 0:1]
                    )
                    si = si_t[:]
                    if t > 0:
                        sf_t = gp.tile([P, FD], F32, name="sf")
                        nc.scalar.activation(
                            sf_t[:], ps[:, FD : 2 * FD], AF.Sigmoid,
                            bias=bl1[:, 1:2],
                        )
                        sf = sf_t[:]
                    so_t = gp.tile([P, FD], BF16, name="so_")
                    nc.scalar.activation(
                        so_t[:], ps[:, 2 * FD : 3 * FD], AF.Sigmoid,
                        bias=bl1[:, 2:3],
                    )
                    so_ = so_t[:]

                if t == 0:
                    nc.vector.tensor_mul(cstate[:], si, tg[:])
                else:
                    t1 = gp.tile([P, FD], F32, name="t1")
                    nc.vector.tensor_mul(t1[:], si, tg[:])
                    t2 = gp.tile([P, FD], F32, name="t2")
                    nc.gpsimd.tensor_mul(t2[:], sf, cstate[:])
                    nc.vector.tensor_add(cstate[:], t1[:], t2[:])

                tcx = gp.tile([P, FD], BF16, name="tcx")
                nc.scalar.activation(tcx[:], cstate[:], AF.Tanh)

                if layer == 0:
                    dst = px.h0[t]
                elif t < 3:
                    dst = px.h1[t]
                else:
                    dst = px.mlph
                nc.vector.tensor_mul(dst[:], so_, tcx[:])

            # ============ phase 1: LSTM stack, two pairs in lockstep ========
            for grp in range((npair + 1) // 2):
                pxs = [open_pair(q) for q in range(2 * grp, min(2 * grp + 2, npair))]
                for layer in (0, 1):
                    for t in range(4):
                        for px in pxs:
                            emit_step(px, layer, t)
                for px in pxs:
                    persist.append((px.mlph, px.ft))

            # ======== phase 2: MLP + log_softmax, two pairs per block =======
            def emit_phase2(block, p0):
                nb = len(block)
                W = 1024 * nb
                ps = pp.tile([128, 2048], F32, name="ps")
                for i, (mlph, ft) in enumerate(block):
                    for half in (0, 1):
                        c = slice(1024 * i + 512 * half, 1024 * i + 512 * half + 512)
                        rows = slice(0, 64) if half == 0 else slice(64, 128)
                        frows = slice(0, 5) if half == 0 else slice(32, 37)
                        nc.tensor.matmul(
                            ps[0:30, c], lhsT=w1h[rows, :], rhs=mlph[rows, :],
                            start=True, stop=False,
                            tile_position=(0 if half == 0 else 64, 0),
                        )
                        nc.tensor.matmul(
                            ps[0:30, c], lhsT=w1f[frows, :], rhs=ft[frows, :],
                            start=False, stop=True,
                            tile_position=(0 if half == 0 else 32, 0),
                        )
                # stages live on disjoint partition rows (0/32/64/96) of the
                # same psum banks, so no bank write-after-read serialization
                m2s = p2.tile([43, 2048], BF16, name="m2s")
                nc.sync.dma_start(m2s[42:43, 0:W], onesbf[:, 0:W])
                m1s = p2.tile([30, 2048], BF16, name="m1s")
                nc.scalar.activation(
                    m1s[:, 0:W], ps[0:30, 0:W], AF.Relu, bias=bm1[:]
                )
                for j in range(2 * nb):
                    c = slice(512 * j, 512 * (j + 1))
                    nc.tensor.matmul(
                        ps[32:42, c], lhsT=wm2[:], rhs=m1s[:, c],
                        start=True, stop=True, tile_position=(0, 32),
                    )
                nc.scalar.activation(
                    m2s[32:42, 0:W], ps[32:42, 0:W], AF.Relu, bias=bm2[32:42, :]
                )
                for j in range(2 * nb):
                    c = slice(512 * j, 512 * (j + 1))
                    nc.tensor.matmul(
                        ps[64:68, c], lhsT=wm3[32:43, :], rhs=m2s[32:43, c],
                        start=True, stop=True, tile_position=(32, 64),
                    )
                es = p2.tile([68, 2048], F32, name="es", bufs=2)
                nc.scalar.activation(es[64:68, 0:W], ps[64:68, 0:W], AF.Exp)
                for j in range(2 * nb):
                    c = slice(512 * j, 512 * (j + 1))
                    nc.tensor.matmul(
                        ps[96:97, c], lhsT=ones4[64:68, :], rhs=es[64:68, c],
                        start=True, stop=True, tile_position=(64, 96),
                    )
                ls = p2.tile([97, 2048], F32, name="ls", bufs=2)
                nc.scalar.activation(ls[96:97, 0:W], ps[96:97, 0:W], AF.Ln)
                for j in range(2 * nb):
                    c = slice(512 * j, 512 * (j + 1))
                    nc.tensor.matmul(
                        ps[64:68, c], lhsT=nones[96:97, :], rhs=ls[96:97, c],
                        start=False, stop=True, tile_position=(96, 64),
                        skip_group_check=True,
                    )
                fo = p2.tile([68, 2048], F32, name="fo", bufs=2)
                nc.vector.tensor_copy(fo[64:68, 0:W], ps[64:68, 0:W])
                nc.sync.dma_start(
                    out_d[:, 1024 * p0 : 1024 * p0 + W], fo[64:68, 0:W]
                )

            for b0 in range(0, npair, 2):
                emit_phase2(persist[b0 : b0 + 2], b0)

    return nc


def pack_weights(Wih0, Whh0, bih0, bhh0, Wih1, Whh1, bih1, bhh1,
                 W1, b1, W2, b2, W3, b3):
    bf = ml_dtypes.bfloat16
    b0 = bih0 + bhh0
    b1l = bih1 + bhh1
    wx6 = np.zeros((6, 512), np.float32)
    wh0 = np.zeros((128, 512), np.float32)
    w1i = np.zeros((128, 512), np.float32)
    w1r = np.zeros((128, 512), np.float32)
    bl1 = np.zeros((128, 4), np.float32)
    for r, sl in enumerate(GSLICE):
        cA = slice(128 * r, 128 * r + 64)
        cB = slice(128 * r + 64, 128 * r + 128)
        wx6[0:2, cA] = Wih0[sl].T
        wx6[2, cA] = b0[sl]
        wx6[3:5, cB] = Wih0[sl].T
        wx6[5, cB] = b0[sl]
        wh0[0:64, cA] = Whh0[sl].T
        wh0[64:128, cB] = Whh0[sl].T
        w1i[0:64, cA] = Wih1[sl].T
        w1i[64:128, cB] = Wih1[sl].T
        w1r[0:64, cA] = Whh1[sl].T
        w1r[64:128, cB] = Whh1[sl].T
        bl1[0:64, r] = b1l[sl]
        bl1[64:128, r] = b1l[sl]
    w1h = np.zeros((128, 30), np.float32)
    w1h[0:64] = W1[:, 0:64].T
    w1h[64:128] = W1[:, 0:64].T
    w1f = np.zeros((37, 30), np.float32)
    w1f[0:5] = W1[:, 64:69].T
    w1f[32:37] = W1[:, 64:69].T
    wm2 = np.ascontiguousarray(W2.T)
    wm3 = np.zeros((43, 4), np.float32)
    wm3[32:42] = W3.T
    wm3[42] = b3
    bm2 = np.zeros((42, 1), np.float32)
    bm2[32:42, 0] = b2
    negones = np.zeros((97, 4), np.float32)
    negones[96] = -1.0
    ones4 = np.zeros((68, 1), np.float32)
    ones4[64:68] = 1.0
    return {
        "wx6": wx6.astype(bf),
        "wh0d": wh0.astype(bf),
        "w1i": w1i.astype(bf),
        "w1r": w1r.astype(bf),
        "w1h": w1h.astype(bf),
        "w1f": w1f.astype(bf),
        "wm2": wm2.astype(bf),
        "wm3": wm3.astype(bf),
        "bl1": bl1,
        "bm1": np.ascontiguousarray(b1.reshape(30, 1), dtype=np.float32),
        "bm2": bm2,
        "negones": negones,
        "ones4": ones4,
    }


def pack_x(xs):
    """xs: [n, 13] f32 -> (xq [4, 6, n//2], fq [5, n]) bf16."""
    n = xs.shape[0]
    npair = n // (2 * FD)
    a = xs.reshape(npair, 2, FD, 13)
    A = a[:, 0].reshape(npair * FD, 13)
    Bv = a[:, 1].reshape(npair * FD, 13)
    xqv = np.zeros((4, 6, npair * FD), np.float32)
    for t in range(4):
        xqv[t, 0:2] = A[:, 2 * t : 2 * t + 2].T
        xqv[t, 2] = 1.0
        xqv[t, 3:5] = Bv[:, 2 * t : 2 * t + 2].T
        xqv[t, 5] = 1.0
    fqv = np.ascontiguousarray(xs[:, 8:13].T)
    return xqv.astype(ml_dtypes.bfloat16), fqv.astype(ml_dtypes.bfloat16)


_cached = {}


def run_cores(x, weights, trace=False):
    """x: [B_TOTAL, 13] f32. Returns (out [B_TOTAL, 4] f32, BassKernelResults)."""
    key = "prog"
    if key not in _cached:
        _cached[key] = build_program(NCHUNK)
    nc = _cached[key]
    in_maps = []
    for c in range(NCORES):
        xs = x[c * B_CORE : (c + 1) * B_CORE]
        m = dict(weights)
        m["xq"], m["fq"] = pack_x(xs)
        in_maps.append(m)
    res = run_bass_kernel_spmd(
        nc, in_maps, core_ids=list(range(NCORES)), trace=trace
    )
    outs = [res.results[c]["out"] for c in range(NCORES)]  # [4, 16384] each
    full = np.concatenate([o.T for o in outs], axis=0)     # [B_TOTAL, 4]
    return np.ascontiguousarray(full, dtype=np.float32), res


def kernel(x, Wih0, Whh0, bih0, bhh0, Wih1, Whh1, bih1, bhh1,
           W1, b1, W2, b2, W3, b3):
    args = [np.asarray(a, dtype=np.float32) for a in (
        Wih0, Whh0, bih0, bhh0, Wih1, Whh1, bih1, bhh1, W1, b1, W2, b2, W3, b3
    )]
    weights = pack_weights(*args)
    out, _ = run_cores(np.asarray(x, dtype=np.float32), weights)
    return out
